# revision 44
# baseline (speedup 1.0000x reference)
"""BalancedTopkMLP Trainium2 kernel: token-parallel across 8 NeuronCores.

reference:
  pred = sigmoid((x @ w_pred1.T) @ w_pred2.T)            [N, I]
  mask = per-bank (128ch) top-16 of |pred|+bias, binary  (bias == 0 here)
  out  = (mask*pred * silu(x@w_gate.T) * (x@w_up.T)) @ w_down.T

Sharding: tokens (B*S = 8192) split 8 ways; each core runs the full MLP on
its 1024 tokens with full weights (no collectives).

Numerics: all matmuls run as fp8(e4m3) DoubleRow-pair matmuls (2 k-slabs
per instruction at 0.5 cycles/row):
  - predictor (both stages): 6-term hi/mid/lo split (3 e4m3 pieces per
    operand, terms (1,1);(2,1),(1,2);(2,2),(1,3),(3,1) accumulated in three
    PSUM scale classes 1/16/256) -> z accurate to ~1e-5 so the per-bank
    top-16 matches the fp32 reference except genuinely near-tied scores.
  - gate/up/down: 3-term split (data 2 pieces, weight 2 pieces as
    pre-scaled e4m3 copies so all 3 terms share one PSUM accumulation).
Selection runs on pre-sigmoid z (monotone; bias==0).
"""
import sys
import os
import numpy as np
import ml_dtypes

for _p in ("/opt/trn_rl_repo", os.path.expanduser("~/.axon_site/_ro/trn_rl_repo")):
    if os.path.isdir(_p) and _p not in sys.path:
        sys.path.insert(0, _p)

import concourse.bass as bass  # noqa: E402
import concourse.mybir as mybir  # noqa: E402
from concourse import bacc  # noqa: E402
from concourse.bass_utils import run_bass_kernel_spmd  # noqa: E402
from concourse.tile import TileContext  # noqa: E402
from concourse.masks import make_identity  # noqa: E402

BF16 = mybir.dt.bfloat16
FP32 = mybir.dt.float32
F32R = mybir.dt.float32r
FP8 = mybir.dt.float8e4
AF = mybir.ActivationFunctionType
OP = mybir.AluOpType
DR = mybir.MatmulPerfMode.DoubleRow

H = 4096
I = 11008
PD = 1024
BANK = 128
TOPK = 16
NB = I // BANK          # 86
NCORES = 8
NTOK_TOT = 8192
NTOK = NTOK_TOT // NCORES   # 1024 per core
KT_H = H // 128             # 32
KT_P = PD // 128            # 8
CB = 4                      # banks per chunk
NCHUNK = (NB + CB - 1) // CB  # 22 (21x4 + 1x2)
NHCG = H // 512             # 8 down-proj H groups
NHC2 = H // 256             # 16 down-proj H groups (256-wide)
NK2 = NB // 2               # 43 k-pairs for down
NEG = -1.0e30

SW1 = 64.0    # w_pred1 scale (sigma 1/64)
SW2 = 32.0    # w_pred2 scale
SG = 64.0     # w_gate / w_up scale
SD = 128.0    # w_down scale

_CACHE = {}
DEBUG = False


def _chunk_banks(ci):
    b0 = ci * CB
    return b0, min(CB, NB - b0)


def _build():
    nc = bacc.Bacc("TRN2", target_bir_lowering=False, debug=False,
                   num_devices=NCORES)

    def din(name, shape):
        return nc.declare_dram_parameter(name, list(shape), FP8, isOutput=False)

    # x pieces: slot0=X2 (16*residual), slot1=X1 (hi)
    xg_d = din("xg", [128, KT_H, 2, NTOK])
    x3_d = din("x3", [128, KT_H, NTOK])        # X3 (256*res2)
    # weights: pieces (W1, W2, W3) = (hi, 16*res, 256*res2) in sigma-scaled space
    w1_d = din("w1", [128, KT_H, 3, PD])
    # w_pred2 hybrid: f32r hi (exact products, clean f32 accumulation) +
    # fp8 pair (e4m3 proxy of hi, e4m3 of 4096*residual) for the correction
    w2h_d = nc.declare_dram_parameter("w2h", [128, KT_P, I], F32R,
                                      isOutput=False)
    w2c_d = din("w2c", [128, KT_P, 2, I])
    # gate/up: slot0 = G1, slot1 = q8(Ws/16), slot2 = q8(Ws - G1)
    wg_d = din("wg", [NB, 128, KT_H, 3, BANK])
    wu_d = din("wu", [NB, 128, KT_H, 3, BANK])
    # down: [hgroup, p, k, slot, 256]; slots (D1, q8(Ws/16), q8(Ws-D1));
    # partition-first so no rearrange DMA is needed. 256-wide H groups so
    # each PSUM bank holds exactly ONE accumulation group at a time
    # (concurrent slice-groups in one bank corrupt accumulation on HW).
    wd_d = din("wd", [NHC2, 128, NB, 3, 256])
    out_d = nc.declare_dram_parameter("out", [NTOK, H], FP32, isOutput=True)
    if DEBUG:
        z_dbg = nc.declare_dram_parameter("z_dbg", [8, 128, 512], FP32,
                                          isOutput=True)
        sg_dbg = nc.declare_dram_parameter("sg_dbg", [128, NTOK], BF16,
                                           isOutput=True)
        uu_dbg = nc.declare_dram_parameter("uu_dbg", [128, NTOK], BF16,
                                           isOutput=True)
        h_dbg = nc.declare_dram_parameter("h_dbg", [128, 2, NTOK], FP8,
                                          isOutput=True)

    from contextlib import ExitStack
    with TileContext(nc) as tc, ExitStack() as es:
        ep = es.enter_context
        constp = ep(tc.tile_pool(name="const", bufs=1))
        dramp = ep(tc.tile_pool(name="dram", bufs=1, space="DRAM"))
        xgp = ep(tc.tile_pool(name="xgp", bufs=1))
        xpp = ep(tc.tile_pool(name="xpp", bufs=1))

        ident = constp.tile([128, 128], BF16)
        make_identity(nc, ident)

        xg = xgp.tile([128, KT_H, 2, NTOK], FP8, tag="xg")
        nc.gpsimd.dma_start(xg[:], xg_d[:])
        # xp hybrid pieces: f32r hi + (e4m3 of 4096*residual, e4m3 proxy of hi)
        xph = xpp.tile([128, KT_P, NTOK], F32R, tag="xph")
        xpc = xpp.tile([128, KT_P, 2, NTOK], FP8, tag="xpc")
        if DEBUG:
            hst = nc.declare_dram_parameter("hst_dbg", [128, NB, 2, NTOK],
                                            FP8, isOutput=True)
        else:
            hst = dramp.tile([128, NB, 2, NTOK], FP8, tag="hst")

        with ExitStack() as es2:
            ep2 = es2.enter_context
            wsp = ep2(tc.tile_pool(name="wsp", bufs=2))
            tmpp = ep2(tc.tile_pool(name="tmpp", bufs=4))
            mmps = ep2(tc.tile_pool(name="mm", bufs=3, space="PSUM"))
            gups = ep2(tc.tile_pool(name="gu", bufs=3, space="PSUM"))
            trps = ep2(tc.tile_pool(name="tr", bufs=2, space="PSUM"))

            def combine_classes(dst, pA, pB, pC, inv_final, cw=512):
                """dst = (pA + pB/16 + pC/256) * inv_final  (dst f32 sbuf)

                GPSIMD cannot read PSUM on HW, so PSUM-reading ops go on
                scalar/vector; the SBUF-only rescale goes on gpsimd."""
                a = tmpp.tile([128, 512], FP32, tag="t")
                nc.scalar.activation(a[:, :cw], pC[:, :cw], AF.Copy,
                                     scale=1.0 / 16)
                b = tmpp.tile([128, 512], FP32, tag="t")
                nc.vector.tensor_tensor(b[:, :cw], a[:, :cw], pB[:, :cw], OP.add)
                c = tmpp.tile([128, 512], FP32, tag="t")
                nc.gpsimd.tensor_scalar_mul(c[:, :cw], b[:, :cw],
                                            inv_final / 16)
                if inv_final != 1.0:
                    d = tmpp.tile([128, 512], FP32, tag="t")
                    nc.scalar.activation(d[:, :cw], pA[:, :cw], AF.Copy,
                                         scale=inv_final)
                    pA = d
                nc.vector.tensor_tensor(dst[:, :cw], c[:, :cw], pA[:, :cw],
                                        OP.add)

            # ---------------- phase 1: xp = x @ w_pred1.T ----------------
            es_ph1 = ExitStack()
            x3p = es_ph1.enter_context(tc.tile_pool(name="x3p", bufs=1))
            for th in range(2):
                t0 = th * 512
                x3t = x3p.tile([128, KT_H, 512], FP8, tag="x3")
                nc.gpsimd.dma_start(x3t[:], x3_d[:, :, t0:t0 + 512])
                for m in range(KT_P):
                    ms = slice(m * 128, (m + 1) * 128)
                    w1m = [wsp.tile([128, KT_H // 2, 3, 128], FP8, tag="ws",
                                    name=f"w1_{th}_{m}_{hh}") for hh in range(2)]
                    for hh in range(2):
                        nc.sync.dma_start(
                            w1m[hh][:], w1_d[:, hh * 16:hh * 16 + 16, :, ms])
                    pA = mmps.tile([128, 512], FP32, tag="mm")
                    pB = mmps.tile([128, 512], FP32, tag="mm")
                    pC = mmps.tile([128, 512], FP32, tag="mm")
                    for tq in range(2):
                        qsl = slice(tq * 256, (tq + 1) * 256)
                        gsl = slice(t0 + tq * 256, t0 + tq * 256 + 256)
                        # class 1: (1,1) as plain fp8 matmuls — DoubleRow
                        # injects ~1e-4 noise per instruction relative to the
                        # accumulator, which the top-16 selection can't absorb
                        # at z scale; plain fp8 accumulates exactly in f32.
                        for k in range(KT_H):
                            w = w1m[k // 16]
                            nc.tensor.matmul(
                                pA[:, qsl], w[:, k % 16, 0, :],
                                xg[:, k, 1, gsl],
                                start=(k == 0), stop=(k == KT_H - 1))
                        # class 16: (2,1)+(1,2) per k
                        for k in range(KT_H):
                            w = w1m[k // 16]
                            nc.tensor.matmul(
                                pB[:, qsl], w[:, k % 16, 0:2, :],
                                xg[:, k, 0:2, gsl],
                                start=(k == 0), stop=(k == KT_H - 1),
                                perf_mode=DR)
                        # class 256: (2,2) pairs; (1,3) pairs; (3,1) pairs
                        for i, (wslot, xslot) in enumerate(
                                ((1, 0), (2, 1), (0, None))):
                            for kp in range(16):
                                w = w1m[kp // 8]
                                k2 = (kp % 8) * 2
                                if xslot is None:
                                    rhs = x3t[:, 2 * kp:2 * kp + 2,
                                              tq * 256:tq * 256 + 256]
                                else:
                                    rhs = xg[:, 2 * kp:2 * kp + 2, xslot, gsl]
                                nc.tensor.matmul(
                                    pC[:, qsl], w[:, k2:k2 + 2, wslot, :], rhs,
                                    start=(i == 0 and kp == 0),
                                    stop=(i == 2 and kp == 15), perf_mode=DR)
                    # combine classes -> true xp; split to 3 e4m3 pieces
                    tsl = slice(t0, t0 + 512)
                    xpf = tmpp.tile([128, 512], FP32, tag="t")
                    combine_classes(xpf, pA, pB, pC, 1.0 / SW1)
                    # f32r hi (rne-12 rounds on write) + residual as fp8
                    nc.scalar.activation(xph[:, m, tsl], xpf[:], AF.Copy)
                    r1 = tmpp.tile([128, 512], FP32, tag="t")
                    nc.vector.tensor_tensor(r1[:], xpf[:],
                                            xph[:, m, tsl].bitcast(FP32),
                                            OP.subtract)
                    nc.scalar.activation(xpc[:, m, 0, tsl], r1[:], AF.Copy,
                                         scale=4096.0)
                    nc.scalar.activation(xpc[:, m, 1, tsl], xpf[:], AF.Copy)

            # ---------------- phase 2: chunks over I ----------------
            es_ph1.close()   # frees the phase-1 x3 stream buffer
            w2p = ep2(tc.tile_pool(name="w2p", bufs=2))
            zp = ep2(tc.tile_pool(name="zp", bufs=2))
            zapp = ep2(tc.tile_pool(name="zapp", bufs=2))
            predp = ep2(tc.tile_pool(name="predp", bufs=2))
            m01p = ep2(tc.tile_pool(name="m01p", bufs=2))
            m8p = ep2(tc.tile_pool(name="m8p", bufs=8))
            mtp = ep2(tc.tile_pool(name="mtp", bufs=1))
            gub = ep2(tc.tile_pool(name="gub", bufs=4))
            htp = ep2(tc.tile_pool(name="htp", bufs=2))
            rp = ep2(tc.tile_pool(name="rp", bufs=2))
            hsp = ep2(tc.tile_pool(name="hsp", bufs=2))

            def gup_unit(b0, b, mat, sg, uu):
                """gate (mat=0) or up (mat=1) for bank b0+b, all 1024 tokens."""
                src = wg_d if mat == 0 else wu_d
                wt = [wsp.tile([128, KT_H // 2, 3, BANK], FP8, tag="ws",
                               name=f"wgu_{b0}_{b}_{mat}_{hh}")
                      for hh in range(2)]
                for hh in range(2):
                    nc.sync.dma_start(wt[hh][:],
                                      src[b0 + b, :, hh * 16:hh * 16 + 16])
                dst = sg if mat == 0 else uu
                for tqp in range(2):
                    pt = gups.tile([128, 512], FP32, tag="gu")
                    for tq in range(2):
                        psl = slice(tq * 256, (tq + 1) * 256)
                        g0 = tqp * 512 + tq * 256
                        gsl = slice(g0, g0 + 256)
                        for kp in range(16):
                            w = wt[kp // 8]
                            k2 = (kp % 8) * 2
                            nc.tensor.matmul(
                                pt[:, psl], w[:, k2:k2 + 2, 0, :],
                                xg[:, 2 * kp:2 * kp + 2, 1, gsl],
                                start=(kp == 0), stop=False, perf_mode=DR)
                        for k in range(KT_H):
                            w = wt[k // 16]
                            nc.tensor.matmul(
                                pt[:, psl], w[:, k % 16, 1:3, :],
                                xg[:, k, 0:2, gsl],
                                start=False, stop=(k == KT_H - 1),
                                perf_mode=DR)
                    nc.scalar.activation(
                        dst[:, tqp * 512:tqp * 512 + 512], pt[:],
                        AF.Silu if mat == 0 else AF.Copy, scale=1.0 / SG)

            for ci in range(NCHUNK):
                b0, nb = _chunk_banks(ci)
                c0, cw = b0 * BANK, nb * BANK
                nhalf = cw // 256
                w2hts, w2cts = [], []
                for hf in range(nhalf):
                    csl = slice(c0 + hf * 256, c0 + hf * 256 + 256)
                    w2ht = w2p.tile([128, KT_P, 256], F32R, tag="w2h",
                                    name=f"w2h_{ci}_{hf}")
                    nc.sync.dma_start(w2ht[:], w2h_d[:, :, csl])
                    w2hts.append(w2ht)
                    w2ct = w2p.tile([128, KT_P, 2, 256], FP8, tag="w2c",
                                    name=f"w2c_{ci}_{hf}")
                    nc.sync.dma_start(w2ct[:], w2c_d[:, :, :, csl])
                    w2cts.append(w2ct)
                units = [(b, mat) for b in range(nb) for mat in range(2)]
                sgs, uus = {}, {}
                for b in range(nb):
                    sgs[b] = gub.tile([128, NTOK], BF16, tag="sg",
                                      name=f"sg_{ci}_{b}")
                    uus[b] = gub.tile([128, NTOK], BF16, tag="uu",
                                      name=f"uu_{ci}_{b}")
                mpT = mtp.tile([128, CB, NTOK], BF16, tag="mpT")
                for tt in range(8):
                    ts = slice(tt * 128, (tt + 1) * 128)
                    pM = mmps.tile([128, 512], FP32, tag="mm")
                    pR = mmps.tile([128, 512], FP32, tag="mm")
                    for hf in range(nhalf):
                        osl = slice(hf * 256, (hf + 1) * 256)
                        # main: f32r hi x hi (exact products, f32 accumulate)
                        for k in range(KT_P):
                            nc.tensor.matmul(
                                pM[:, osl], xph[:, k, ts],
                                w2hts[hf][:, k, :],
                                start=(k == 0), stop=(k == KT_P - 1))
                        # correction: xl*hi + hi*w2l (both scaled 4096), DR
                        for k in range(KT_P):
                            nc.tensor.matmul(
                                pR[:, osl], xpc[:, k, 0:2, ts],
                                w2cts[hf][:, k, :, :],
                                start=(k == 0), stop=(k == KT_P - 1),
                                perf_mode=DR)
                    # interleave one gate/up unit per tt to keep PE busy
                    # while z(tt) goes through combine/select on vector+scalar
                    if tt < len(units):
                        ub, umat = units[tt]
                        gup_unit(b0, ub, umat, sgs[ub], uus[ub])
                    # z_s = 32*z; selection is scale-invariant
                    zt = zp.tile([128, 512], FP32, tag="z")
                    az = tmpp.tile([128, 512], FP32, tag="t")
                    nc.scalar.activation(az[:, :cw], pR[:, :cw], AF.Copy,
                                         scale=1.0 / 4096)
                    nc.vector.tensor_tensor(zt[:, :cw], az[:, :cw],
                                            pM[:, :cw], OP.add)
                    if DEBUG and ci == 0:
                        nc.sync.dma_start(z_dbg[tt], zt[:])
                    pred = predp.tile([128, 512], BF16, tag="pred")
                    nc.scalar.activation(pred[:, :cw], zt[:, :cw], AF.Sigmoid,
                                         scale=1.0 / SW2)
                    zap = zapp.tile([128, 512], FP32, tag="zap")
                    for b in range(nb):
                        bs = slice(b * BANK, (b + 1) * BANK)
                        m8 = m8p.tile([128, 8], FP32, tag="m8")
                        nc.vector.max(m8[:], zt[:, bs])
                        nc.vector.match_replace(zap[:, bs], in_to_replace=m8[:],
                                                in_values=zt[:, bs],
                                                imm_value=NEG)
                        m8b = m8p.tile([128, 8], FP32, tag="m8")
                        nc.vector.max(m8b[:], zap[:, bs])
                        nc.vector.match_replace(zap[:, bs],
                                                in_to_replace=m8b[:],
                                                in_values=zap[:, bs],
                                                imm_value=NEG)
                    m01 = m01p.tile([128, 512], BF16, tag="m01")
                    nc.vector.tensor_tensor(m01[:, :cw], zt[:, :cw],
                                            zap[:, :cw], OP.not_equal)
                    nc.vector.tensor_tensor(pred[:, :cw], m01[:, :cw],
                                            pred[:, :cw], OP.mult)
                    # transposes after the gup unit (pred ready by then)
                    for b in range(nb):
                        bs = slice(b * BANK, (b + 1) * BANK)
                        tp = trps.tile([128, 128], BF16, tag="tr")
                        nc.tensor.transpose(tp[:], pred[:, bs], ident[:])
                        nc.scalar.activation(mpT[:, b, ts], tp[:], AF.Copy)
                # remaining gup units (ragged last chunk)
                for ui in range(8, len(units)):
                    ub, umat = units[ui]
                    gup_unit(b0, ub, umat, sgs[ub], uus[ub])
                # h = masked_pred * silu(gate) * up -> 2-piece e4m3 stash
                for b in range(nb):
                    hsts = hsp.tile([128, 2, NTOK], FP8, tag="hs",
                                    name=f"hs_{ci}_{b}")
                    for hh in range(2):
                        hsl = slice(hh * 512, (hh + 1) * 512)
                        htf = htp.tile([128, 512], FP32, tag="htf")
                        nc.vector.tensor_tensor(htf[:], mpT[:, b, hsl],
                                                sgs[b][:, hsl], OP.mult)
                        nc.vector.tensor_tensor(htf[:], htf[:],
                                                uus[b][:, hsl], OP.mult)
                        nc.scalar.activation(hsts[:, 1, hsl], htf[:], AF.Copy)
                        r = rp.tile([128, 512], FP32, tag="r")
                        nc.vector.tensor_tensor(r[:], htf[:], hsts[:, 1, hsl],
                                                OP.subtract)
                        nc.scalar.activation(hsts[:, 0, hsl], r[:], AF.Copy,
                                             scale=16.0)
                    nc.sync.dma_start(hst[:, b0 + b, :, :], hsts[:])
                    if DEBUG and ci == 0 and b == 0:
                        nc.sync.dma_start(sg_dbg[:], sgs[0][:])
                        nc.sync.dma_start(uu_dbg[:], uus[0][:])
                        nc.sync.dma_start(h_dbg[:], hsts[:])

        # ---------------- phase 3: out = h @ w_down.T ----------------
        with ExitStack() as es3:
            ep3 = es3.enter_context
            dnp = ep3(tc.tile_pool(name="dnp", bufs=3))
            hsbp = ep3(tc.tile_pool(name="hsbp", bufs=1))
            osp = ep3(tc.tile_pool(name="osp", bufs=2))
            dnps = ep3(tc.tile_pool(name="dn", bufs=8, space="PSUM"))
            for th in range(2):
                t0 = th * 512
                # h pieces for this token half stay SBUF-resident: down is
                # then weight-stream-bound only (wd read twice total).
                hsb = hsbp.tile([128, NB, 2, 512], FP8, tag="hsb")
                nc.gpsimd.dma_start(hsb[:, :NB // 2],
                                    hst[:, :NB // 2, :, t0:t0 + 512])
                nc.sync.dma_start(hsb[:, NB // 2:],
                                  hst[:, NB // 2:, :, t0:t0 + 512])
                for hg in range(NHC2):
                    # [128, 512] psum tiles are bank-granular; only [:, :256]
                    # is used so each bank hosts ONE accumulation group
                    # (concurrent slice-groups in one bank corrupt on HW).
                    pts = [dnps.tile([128, 512], FP32, tag="dn",
                                     name=f"dn_{th}_{hg}_{t}")
                           for t in range(4)]
                    for k2 in range(NK2):
                        wdm = dnp.tile([128, 2, 256], FP8, tag="wdm")
                        nc.sync.dma_start(
                            wdm[:], wd_d[hg, :, 2 * k2:2 * k2 + 2, 0, :])
                        wdc = dnp.tile([128, 2, 2, 256], FP8, tag="wdc")
                        nc.gpsimd.dma_start(
                            wdc[:], wd_d[hg, :, 2 * k2:2 * k2 + 2, 1:3, :])
                        for t4 in range(4):
                            ts = slice(t4 * 128, (t4 + 1) * 128)
                            nc.tensor.matmul(
                                pts[t4][:, 0:256],
                                hsb[:, 2 * k2:2 * k2 + 2, 1, ts],
                                wdm[:, :, :],
                                start=(k2 == 0), stop=False, perf_mode=DR)
                            for kk in range(2):
                                nc.tensor.matmul(
                                    pts[t4][:, 0:256],
                                    hsb[:, 2 * k2 + kk, 0:2, ts],
                                    wdc[:, kk, :, :],
                                    start=False,
                                    stop=(k2 == NK2 - 1 and kk == 1),
                                    perf_mode=DR)
                    for t4 in range(4):
                        ot = osp.tile([128, 256], FP32, tag="os")
                        nc.scalar.activation(ot[:], pts[t4][:, 0:256],
                                             AF.Copy, scale=1.0 / SD)
                        tg = th * 4 + t4
                        nc.sync.dma_start(
                            out_d[tg * 128:(tg + 1) * 128,
                                  hg * 256:(hg + 1) * 256], ot[:])

    nc.compile()
    return nc


F8NP = ml_dtypes.float8_e4m3


def _q8(a):
    return a.astype(F8NP)


def _rne12(a):
    """float32r rounding: round-to-nearest-even keeping 11 explicit mantissa
    bits (drops 12 low bits), as measured on TRN2 via identity matmul."""
    v = np.ascontiguousarray(a, np.float32).view(np.uint32)
    add = np.uint32((1 << 11) - 1)
    lsb = (v >> np.uint32(12)) & np.uint32(1)
    return ((v + add + lsb) & np.uint32(0xFFFFF000)).view(np.float32)


def _split3(a):
    """3-piece e4m3 split: a ~ p1 + p2/16 + p3/256."""
    p1 = _q8(a)
    r1 = a - p1.astype(np.float32)
    p2 = _q8(16.0 * r1)
    r2 = r1 - p2.astype(np.float32) / 16.0
    p3 = _q8(256.0 * r2)
    return p1, p2, p3


def _tile_k(a, kt):
    """[K, N] -> [128, kt, N]"""
    K, N = a.shape
    return np.ascontiguousarray(a.reshape(kt, 128, N).transpose(1, 0, 2))


def _prep_weights(w_pred1, w_pred2, w_gate, w_up, w_down):
    # predictor pieces: [128, kt, 3, N]
    def pred_pieces(wT, kt, scale):
        p1, p2, p3 = _split3(wT * scale)
        return np.ascontiguousarray(
            np.stack([_tile_k(p1, kt), _tile_k(p2, kt), _tile_k(p3, kt)],
                     axis=2))

    w1 = pred_pieces(w_pred1.T.copy(), KT_H, SW1)      # [128,32,3,PD]
    # w2 hybrid: f32r hi + fp8 correction pair
    w2s = w_pred2.T.copy() * SW2                        # [P, I]
    w2hi = _rne12(w2s)
    w2h = _tile_k(w2hi, KT_P)                           # [128,8,I] f32 (F32R)
    w2c = np.ascontiguousarray(np.stack(
        [_tile_k(_q8(w2s).astype(np.float32), KT_P),
         _tile_k(_q8(4096.0 * (w2s - w2hi)).astype(np.float32), KT_P)],
        axis=2).astype(F8NP))                           # [128,8,2,I]

    def gu_pieces(wT, scale):
        ws = wT * scale                                # [H, I]
        g1 = _q8(ws)
        g1_16 = _q8(ws / 16.0)
        gr_16 = _q8(ws - g1.astype(np.float32))
        # [NB, 128, KT_H, 3, BANK]
        def lay(a):
            return a.reshape(KT_H, 128, NB, BANK).transpose(2, 1, 0, 3)
        return np.ascontiguousarray(
            np.stack([lay(g1), lay(g1_16), lay(gr_16)], axis=3))

    wg = gu_pieces(w_gate.T.copy(), SG)
    wu = gu_pieces(w_up.T.copy(), SG)

    ws = w_down.T.copy() * SD                          # [I, H]
    d1 = _q8(ws)
    d1_16 = _q8(ws / 16.0)
    dr_16 = _q8(ws - d1.astype(np.float32))
    # [NHC2, 128, NB, 3, 256] (partition-first)
    def dlay(a):
        return a.reshape(NB, 128, NHC2, 256).transpose(2, 1, 0, 3)
    wd = np.ascontiguousarray(
        np.stack([dlay(d1), dlay(d1_16), dlay(dr_16)], axis=3))
    return {"w1": w1, "w2h": w2h, "w2c": w2c, "wg": wg, "wu": wu, "wd": wd}


def _prep_inputs(x, w_pred1, w_pred2, w_gate, w_up, w_down):
    shared = _prep_weights(w_pred1, w_pred2, w_gate, w_up, w_down)
    x2 = x.reshape(NTOK_TOT, H)
    maps = []
    for c in range(NCORES):
        xT = x2[c * NTOK:(c + 1) * NTOK].T.copy()      # [H, NTOK]
        p1, p2, p3 = _split3(xT)
        m = dict(shared)
        m["xg"] = np.ascontiguousarray(
            np.stack([_tile_k(p2, KT_H), _tile_k(p1, KT_H)], axis=2))
        m["x3"] = _tile_k(p3, KT_H)
        maps.append(m)
    return maps


def kernel(x, w_pred1, w_pred2, w_gate, w_up, w_down, balanced_bias,
           trace=False):
    x = np.asarray(x, dtype=np.float32)
    assert not np.any(np.asarray(balanced_bias)), \
        "kernel assumes balanced_bias == 0 (as produced by setup_inputs)"
    if "nc" not in _CACHE:
        _CACHE["nc"] = _build()
    nc = _CACHE["nc"]
    maps = _prep_inputs(x, np.asarray(w_pred1, np.float32),
                        np.asarray(w_pred2, np.float32),
                        np.asarray(w_gate, np.float32),
                        np.asarray(w_up, np.float32),
                        np.asarray(w_down, np.float32))
    res = run_bass_kernel_spmd(nc, maps, list(range(NCORES)), trace=trace)
    out = np.concatenate([res.results[c]["out"] for c in range(NCORES)], axis=0)
    out = out.reshape(x.shape[0], x.shape[1], H)
    if trace:
        _CACHE["last_result"] = res
    return out


# revision 46
# speedup vs baseline: 1.1035x; 1.1035x over previous
"""BalancedTopkMLP Trainium2 kernel: token-parallel across 8 NeuronCores.

reference:
  pred = sigmoid((x @ w_pred1.T) @ w_pred2.T)            [N, I]
  mask = per-bank (128ch) top-16 of |pred|+bias, binary  (bias == 0 here)
  out  = (mask*pred * silu(x@w_gate.T) * (x@w_up.T)) @ w_down.T

Sharding: tokens (B*S = 8192) split 8 ways; each core runs the full MLP on
its 1024 tokens with full weights (no collectives).

Numerics: all matmuls run as fp8(e4m3) DoubleRow-pair matmuls (2 k-slabs
per instruction at 0.5 cycles/row):
  - predictor (both stages): 6-term hi/mid/lo split (3 e4m3 pieces per
    operand, terms (1,1);(2,1),(1,2);(2,2),(1,3),(3,1) accumulated in three
    PSUM scale classes 1/16/256) -> z accurate to ~1e-5 so the per-bank
    top-16 matches the fp32 reference except genuinely near-tied scores.
  - gate/up/down: 3-term split (data 2 pieces, weight 2 pieces as
    pre-scaled e4m3 copies so all 3 terms share one PSUM accumulation).
Selection runs on pre-sigmoid z (monotone; bias==0).
"""
import sys
import os
import numpy as np
import ml_dtypes

for _p in ("/opt/trn_rl_repo", os.path.expanduser("~/.axon_site/_ro/trn_rl_repo")):
    if os.path.isdir(_p) and _p not in sys.path:
        sys.path.insert(0, _p)

import concourse.bass as bass  # noqa: E402
import concourse.mybir as mybir  # noqa: E402
from concourse import bacc  # noqa: E402
from concourse.bass_utils import run_bass_kernel_spmd  # noqa: E402
from concourse.tile import TileContext  # noqa: E402
from concourse.masks import make_identity  # noqa: E402

BF16 = mybir.dt.bfloat16
FP32 = mybir.dt.float32
F32R = mybir.dt.float32r
FP8 = mybir.dt.float8e4
AF = mybir.ActivationFunctionType
OP = mybir.AluOpType
DR = mybir.MatmulPerfMode.DoubleRow

H = 4096
I = 11008
PD = 1024
BANK = 128
TOPK = 16
NB = I // BANK          # 86
NCORES = 8
NTOK_TOT = 8192
NTOK = NTOK_TOT // NCORES   # 1024 per core
KT_H = H // 128             # 32
KT_P = PD // 128            # 8
CB = 4                      # banks per chunk
NCHUNK = (NB + CB - 1) // CB  # 22 (21x4 + 1x2)
NHCG = H // 512             # 8 down-proj H groups
NHC2 = H // 256             # 16 down-proj H groups (256-wide)
NK2 = NB // 2               # 43 k-pairs for down
NEG = -1.0e30

SW1 = 64.0    # w_pred1 scale (sigma 1/64)
SW2 = 32.0    # w_pred2 scale
SG = 64.0     # w_gate / w_up scale
SD = 128.0    # w_down scale

_CACHE = {}
DEBUG = False


def _chunk_banks(ci):
    b0 = ci * CB
    return b0, min(CB, NB - b0)


def _build():
    nc = bacc.Bacc("TRN2", target_bir_lowering=False, debug=False,
                   num_devices=NCORES)

    def din(name, shape):
        return nc.declare_dram_parameter(name, list(shape), FP8, isOutput=False)

    # x pieces: slot0=X2 (16*residual), slot1=X1 (hi)
    xg_d = din("xg", [128, KT_H, 2, NTOK])
    x3_d = din("x3", [128, KT_H, NTOK])        # X3 (256*res2)
    # weights: pieces (W1, W2, W3) = (hi, 16*res, 256*res2) in sigma-scaled space
    w1_d = din("w1", [128, KT_H, 3, PD])
    # w_pred2 hybrid: f32r hi (exact products, clean f32 accumulation) +
    # fp8 pair (e4m3 proxy of hi, e4m3 of 4096*residual) for the correction
    w2h_d = nc.declare_dram_parameter("w2h", [128, KT_P, I], F32R,
                                      isOutput=False)
    w2c_d = din("w2c", [128, KT_P, 2, I])
    # gate/up: slot0 = G1, slot1 = q8(Ws/16), slot2 = q8(Ws - G1)
    wg_d = din("wg", [NB, 128, KT_H, 3, BANK])
    wu_d = din("wu", [NB, 128, KT_H, 3, BANK])
    # down: [hgroup, p, k, slot, 256]; slots (D1, q8(Ws/16), q8(Ws-D1));
    # partition-first so no rearrange DMA is needed. 256-wide H groups so
    # each PSUM bank holds exactly ONE accumulation group at a time
    # (concurrent slice-groups in one bank corrupt accumulation on HW).
    wd_d = din("wd", [NHC2, 128, NB, 3, 256])
    out_d = nc.declare_dram_parameter("out", [NTOK, H], FP32, isOutput=True)
    if DEBUG:
        z_dbg = nc.declare_dram_parameter("z_dbg", [8, 128, 512], FP32,
                                          isOutput=True)
        sg_dbg = nc.declare_dram_parameter("sg_dbg", [128, NTOK], BF16,
                                           isOutput=True)
        uu_dbg = nc.declare_dram_parameter("uu_dbg", [128, NTOK], BF16,
                                           isOutput=True)
        h_dbg = nc.declare_dram_parameter("h_dbg", [128, 2, NTOK], FP8,
                                          isOutput=True)

    from contextlib import ExitStack
    with TileContext(nc) as tc, ExitStack() as es:
        ep = es.enter_context
        constp = ep(tc.tile_pool(name="const", bufs=1))
        dramp = ep(tc.tile_pool(name="dram", bufs=1, space="DRAM"))
        xgp = ep(tc.tile_pool(name="xgp", bufs=1))
        xpp = ep(tc.tile_pool(name="xpp", bufs=1))

        ident = constp.tile([128, 128], BF16)
        make_identity(nc, ident)

        xg = xgp.tile([128, KT_H, 2, NTOK], FP8, tag="xg")
        nc.gpsimd.dma_start(xg[:], xg_d[:])
        # xp hybrid pieces: f32r hi + (e4m3 of 4096*residual, e4m3 proxy of hi)
        xph = xpp.tile([128, KT_P, NTOK], F32R, tag="xph")
        xpc = xpp.tile([128, KT_P, 2, NTOK], FP8, tag="xpc")
        if DEBUG:
            hst = nc.declare_dram_parameter("hst_dbg", [128, NB, 2, NTOK],
                                            FP8, isOutput=True)
        else:
            hst = dramp.tile([128, NB, 2, NTOK], FP8, tag="hst")

        with ExitStack() as es2:
            ep2 = es2.enter_context
            wsp = ep2(tc.tile_pool(name="wsp", bufs=3))
            tmpp = ep2(tc.tile_pool(name="tmpp", bufs=4))
            mmps = ep2(tc.tile_pool(name="mm", bufs=3, space="PSUM"))
            gups = ep2(tc.tile_pool(name="gu", bufs=3, space="PSUM"))
            trps = ep2(tc.tile_pool(name="tr", bufs=2, space="PSUM"))

            def combine_classes(dst, pA, pB, pC, inv_final, cw=512):
                """dst = (pA + pB/16 + pC/256) * inv_final  (dst f32 sbuf)

                GPSIMD cannot read PSUM on HW, so PSUM-reading ops go on
                scalar/vector; the SBUF-only rescale goes on gpsimd."""
                a = tmpp.tile([128, 512], FP32, tag="t")
                nc.scalar.activation(a[:, :cw], pC[:, :cw], AF.Copy,
                                     scale=1.0 / 16)
                b = tmpp.tile([128, 512], FP32, tag="t")
                nc.vector.tensor_tensor(b[:, :cw], a[:, :cw], pB[:, :cw], OP.add)
                c = tmpp.tile([128, 512], FP32, tag="t")
                nc.gpsimd.tensor_scalar_mul(c[:, :cw], b[:, :cw],
                                            inv_final / 16)
                if inv_final != 1.0:
                    d = tmpp.tile([128, 512], FP32, tag="t")
                    nc.scalar.activation(d[:, :cw], pA[:, :cw], AF.Copy,
                                         scale=inv_final)
                    pA = d
                nc.vector.tensor_tensor(dst[:, :cw], c[:, :cw], pA[:, :cw],
                                        OP.add)

            # ---------------- phase 1: xp = x @ w_pred1.T ----------------
            es_ph1 = ExitStack()
            x3p = es_ph1.enter_context(tc.tile_pool(name="x3p", bufs=1))
            for th in range(2):
                t0 = th * 512
                x3t = x3p.tile([128, KT_H, 512], FP8, tag="x3")
                nc.gpsimd.dma_start(x3t[:], x3_d[:, :, t0:t0 + 512])
                for m in range(KT_P):
                    ms = slice(m * 128, (m + 1) * 128)
                    w1m = [wsp.tile([128, KT_H // 2, 3, 128], FP8, tag="ws",
                                    name=f"w1_{th}_{m}_{hh}") for hh in range(2)]
                    for hh in range(2):
                        nc.sync.dma_start(
                            w1m[hh][:], w1_d[:, hh * 16:hh * 16 + 16, :, ms])
                    pA = mmps.tile([128, 512], FP32, tag="mm")
                    pB = mmps.tile([128, 512], FP32, tag="mm")
                    pC = mmps.tile([128, 512], FP32, tag="mm")
                    for tq in range(2):
                        qsl = slice(tq * 256, (tq + 1) * 256)
                        gsl = slice(t0 + tq * 256, t0 + tq * 256 + 256)
                        # class 1: (1,1) as plain fp8 matmuls — DoubleRow
                        # injects ~1e-4 noise per instruction relative to the
                        # accumulator, which the top-16 selection can't absorb
                        # at z scale; plain fp8 accumulates exactly in f32.
                        for k in range(KT_H):
                            w = w1m[k // 16]
                            nc.tensor.matmul(
                                pA[:, qsl], w[:, k % 16, 0, :],
                                xg[:, k, 1, gsl],
                                start=(k == 0), stop=(k == KT_H - 1))
                        # class 16: (2,1)+(1,2) per k
                        for k in range(KT_H):
                            w = w1m[k // 16]
                            nc.tensor.matmul(
                                pB[:, qsl], w[:, k % 16, 0:2, :],
                                xg[:, k, 0:2, gsl],
                                start=(k == 0), stop=(k == KT_H - 1),
                                perf_mode=DR)
                        # class 256: (2,2) pairs; (1,3) pairs; (3,1) pairs
                        for i, (wslot, xslot) in enumerate(
                                ((1, 0), (2, 1), (0, None))):
                            for kp in range(16):
                                w = w1m[kp // 8]
                                k2 = (kp % 8) * 2
                                if xslot is None:
                                    rhs = x3t[:, 2 * kp:2 * kp + 2,
                                              tq * 256:tq * 256 + 256]
                                else:
                                    rhs = xg[:, 2 * kp:2 * kp + 2, xslot, gsl]
                                nc.tensor.matmul(
                                    pC[:, qsl], w[:, k2:k2 + 2, wslot, :], rhs,
                                    start=(i == 0 and kp == 0),
                                    stop=(i == 2 and kp == 15), perf_mode=DR)
                    # combine classes -> true xp; split to 3 e4m3 pieces
                    tsl = slice(t0, t0 + 512)
                    xpf = tmpp.tile([128, 512], FP32, tag="t")
                    combine_classes(xpf, pA, pB, pC, 1.0 / SW1)
                    # f32r hi (rne-12 rounds on write) + residual as fp8
                    nc.scalar.activation(xph[:, m, tsl], xpf[:], AF.Copy)
                    r1 = tmpp.tile([128, 512], FP32, tag="t")
                    nc.vector.tensor_tensor(r1[:], xpf[:],
                                            xph[:, m, tsl].bitcast(FP32),
                                            OP.subtract)
                    nc.scalar.activation(xpc[:, m, 0, tsl], r1[:], AF.Copy,
                                         scale=4096.0)
                    nc.scalar.activation(xpc[:, m, 1, tsl], xpf[:], AF.Copy)

            # ---------------- phase 2: chunks over I ----------------
            es_ph1.close()   # frees the phase-1 x3 stream buffer
            w2p = ep2(tc.tile_pool(name="w2p", bufs=2))
            zp = ep2(tc.tile_pool(name="zp", bufs=2))
            zapp = ep2(tc.tile_pool(name="zapp", bufs=1))
            predp = ep2(tc.tile_pool(name="predp", bufs=2))
            m01p = ep2(tc.tile_pool(name="m01p", bufs=1))
            m8p = ep2(tc.tile_pool(name="m8p", bufs=4))
            mtp = ep2(tc.tile_pool(name="mtp", bufs=1))
            gub = ep2(tc.tile_pool(name="gub", bufs=4))
            htp = ep2(tc.tile_pool(name="htp", bufs=2))
            rp = ep2(tc.tile_pool(name="rp", bufs=2))
            hsp = ep2(tc.tile_pool(name="hsp", bufs=2))

            def gup_unit(b0, b, mat, sg, uu):
                """gate (mat=0) or up (mat=1) for bank b0+b, all 1024 tokens."""
                src = wg_d if mat == 0 else wu_d
                wt = [wsp.tile([128, KT_H // 2, 3, BANK], FP8, tag="ws",
                               name=f"wgu_{b0}_{b}_{mat}_{hh}")
                      for hh in range(2)]
                for hh in range(2):
                    nc.sync.dma_start(wt[hh][:],
                                      src[b0 + b, :, hh * 16:hh * 16 + 16])
                dst = sg if mat == 0 else uu
                for tqp in range(2):
                    pt = gups.tile([128, 512], FP32, tag="gu")
                    for tq in range(2):
                        psl = slice(tq * 256, (tq + 1) * 256)
                        g0 = tqp * 512 + tq * 256
                        gsl = slice(g0, g0 + 256)
                        for kp in range(16):
                            w = wt[kp // 8]
                            k2 = (kp % 8) * 2
                            nc.tensor.matmul(
                                pt[:, psl], w[:, k2:k2 + 2, 0, :],
                                xg[:, 2 * kp:2 * kp + 2, 1, gsl],
                                start=(kp == 0), stop=False, perf_mode=DR)
                        for k in range(KT_H):
                            w = wt[k // 16]
                            nc.tensor.matmul(
                                pt[:, psl], w[:, k % 16, 1:3, :],
                                xg[:, k, 0:2, gsl],
                                start=False, stop=(k == KT_H - 1),
                                perf_mode=DR)
                    nc.scalar.activation(
                        dst[:, tqp * 512:tqp * 512 + 512], pt[:],
                        AF.Silu if mat == 0 else AF.Copy, scale=1.0 / SG)

            for ci in range(NCHUNK):
                b0, nb = _chunk_banks(ci)
                c0, cw = b0 * BANK, nb * BANK
                nhalf = cw // 256
                w2hts, w2cts = [], []
                for hf in range(nhalf):
                    csl = slice(c0 + hf * 256, c0 + hf * 256 + 256)
                    w2ht = w2p.tile([128, KT_P, 256], F32R, tag="w2h",
                                    name=f"w2h_{ci}_{hf}")
                    nc.sync.dma_start(w2ht[:], w2h_d[:, :, csl])
                    w2hts.append(w2ht)
                    w2ct = w2p.tile([128, KT_P, 2, 256], FP8, tag="w2c",
                                    name=f"w2c_{ci}_{hf}")
                    nc.sync.dma_start(w2ct[:], w2c_d[:, :, :, csl])
                    w2cts.append(w2ct)
                units = [(b, mat) for b in range(nb) for mat in range(2)]
                sgs, uus = {}, {}
                for b in range(nb):
                    sgs[b] = gub.tile([128, NTOK], BF16, tag="sg",
                                      name=f"sg_{ci}_{b}")
                    uus[b] = gub.tile([128, NTOK], BF16, tag="uu",
                                      name=f"uu_{ci}_{b}")
                mpT = mtp.tile([128, CB, NTOK], BF16, tag="mpT")
                for tt in range(8):
                    ts = slice(tt * 128, (tt + 1) * 128)
                    pM = mmps.tile([128, 512], FP32, tag="mm")
                    pR = mmps.tile([128, 512], FP32, tag="mm")
                    for hf in range(nhalf):
                        osl = slice(hf * 256, (hf + 1) * 256)
                        # main: f32r hi x hi (exact products, f32 accumulate)
                        for k in range(KT_P):
                            nc.tensor.matmul(
                                pM[:, osl], xph[:, k, ts],
                                w2hts[hf][:, k, :],
                                start=(k == 0), stop=(k == KT_P - 1))
                        # correction: xl*hi + hi*w2l (both scaled 4096), DR
                        for k in range(KT_P):
                            nc.tensor.matmul(
                                pR[:, osl], xpc[:, k, 0:2, ts],
                                w2cts[hf][:, k, :, :],
                                start=(k == 0), stop=(k == KT_P - 1),
                                perf_mode=DR)
                    # interleave one gate/up unit per tt to keep PE busy
                    # while z(tt) goes through combine/select on vector+scalar
                    if tt < len(units):
                        ub, umat = units[tt]
                        gup_unit(b0, ub, umat, sgs[ub], uus[ub])
                    # z_s = 32*z; selection is scale-invariant
                    zt = zp.tile([128, 512], FP32, tag="z")
                    az = tmpp.tile([128, 512], FP32, tag="t")
                    nc.scalar.activation(az[:, :cw], pR[:, :cw], AF.Copy,
                                         scale=1.0 / 4096)
                    nc.vector.tensor_tensor(zt[:, :cw], az[:, :cw],
                                            pM[:, :cw], OP.add)
                    if DEBUG and ci == 0:
                        nc.sync.dma_start(z_dbg[tt], zt[:])
                    pred = predp.tile([128, 512], BF16, tag="pred")
                    nc.scalar.activation(pred[:, :cw], zt[:, :cw], AF.Sigmoid,
                                         scale=1.0 / SW2)
                    zap = zapp.tile([128, 512], FP32, tag="zap")
                    for b in range(nb):
                        bs = slice(b * BANK, (b + 1) * BANK)
                        m8 = m8p.tile([128, 8], FP32, tag="m8")
                        nc.vector.max(m8[:], zt[:, bs])
                        nc.vector.match_replace(zap[:, bs], in_to_replace=m8[:],
                                                in_values=zt[:, bs],
                                                imm_value=NEG)
                        m8b = m8p.tile([128, 8], FP32, tag="m8")
                        nc.vector.max(m8b[:], zap[:, bs])
                        nc.vector.match_replace(zap[:, bs],
                                                in_to_replace=m8b[:],
                                                in_values=zap[:, bs],
                                                imm_value=NEG)
                    m01 = m01p.tile([128, 512], BF16, tag="m01")
                    nc.vector.tensor_tensor(m01[:, :cw], zt[:, :cw],
                                            zap[:, :cw], OP.not_equal)
                    nc.vector.tensor_tensor(pred[:, :cw], m01[:, :cw],
                                            pred[:, :cw], OP.mult)
                    # transposes after the gup unit (pred ready by then)
                    for b in range(nb):
                        bs = slice(b * BANK, (b + 1) * BANK)
                        tp = trps.tile([128, 128], BF16, tag="tr")
                        nc.tensor.transpose(tp[:], pred[:, bs], ident[:])
                        nc.scalar.activation(mpT[:, b, ts], tp[:], AF.Copy)
                # remaining gup units (ragged last chunk)
                for ui in range(8, len(units)):
                    ub, umat = units[ui]
                    gup_unit(b0, ub, umat, sgs[ub], uus[ub])
                # h = masked_pred * silu(gate) * up -> 2-piece e4m3 stash
                for b in range(nb):
                    hsts = hsp.tile([128, 2, NTOK], FP8, tag="hs",
                                    name=f"hs_{ci}_{b}")
                    for hh in range(2):
                        hsl = slice(hh * 512, (hh + 1) * 512)
                        htf = htp.tile([128, 512], FP32, tag="htf")
                        nc.vector.tensor_tensor(htf[:], mpT[:, b, hsl],
                                                sgs[b][:, hsl], OP.mult)
                        nc.vector.tensor_tensor(htf[:], htf[:],
                                                uus[b][:, hsl], OP.mult)
                        nc.scalar.activation(hsts[:, 1, hsl], htf[:], AF.Copy)
                        r = rp.tile([128, 512], FP32, tag="r")
                        nc.vector.tensor_tensor(r[:], htf[:], hsts[:, 1, hsl],
                                                OP.subtract)
                        nc.scalar.activation(hsts[:, 0, hsl], r[:], AF.Copy,
                                             scale=16.0)
                    nc.sync.dma_start(hst[:, b0 + b, :, :], hsts[:])
                    if DEBUG and ci == 0 and b == 0:
                        nc.sync.dma_start(sg_dbg[:], sgs[0][:])
                        nc.sync.dma_start(uu_dbg[:], uus[0][:])
                        nc.sync.dma_start(h_dbg[:], hsts[:])

        # ---------------- phase 3: out = h @ w_down.T ----------------
        with ExitStack() as es3:
            ep3 = es3.enter_context
            dnp = ep3(tc.tile_pool(name="dnp", bufs=3))
            hsbp = ep3(tc.tile_pool(name="hsbp", bufs=1))
            osp = ep3(tc.tile_pool(name="osp", bufs=2))
            dnps = ep3(tc.tile_pool(name="dn", bufs=8, space="PSUM"))
            for th in range(2):
                t0 = th * 512
                # h pieces for this token half stay SBUF-resident: down is
                # then weight-stream-bound only (wd read twice total).
                hsb = hsbp.tile([128, NB, 2, 512], FP8, tag="hsb")
                nc.gpsimd.dma_start(hsb[:, :NB // 2],
                                    hst[:, :NB // 2, :, t0:t0 + 512])
                nc.sync.dma_start(hsb[:, NB // 2:],
                                  hst[:, NB // 2:, :, t0:t0 + 512])
                for hg in range(NHC2):
                    # [128, 512] psum tiles are bank-granular; only [:, :256]
                    # is used so each bank hosts ONE accumulation group
                    # (concurrent slice-groups in one bank corrupt on HW).
                    pts = [dnps.tile([128, 512], FP32, tag="dn",
                                     name=f"dn_{th}_{hg}_{t}")
                           for t in range(4)]
                    for k2 in range(NK2):
                        wdm = dnp.tile([128, 2, 256], FP8, tag="wdm")
                        nc.sync.dma_start(
                            wdm[:], wd_d[hg, :, 2 * k2:2 * k2 + 2, 0, :])
                        wdc = dnp.tile([128, 2, 2, 256], FP8, tag="wdc")
                        nc.gpsimd.dma_start(
                            wdc[:], wd_d[hg, :, 2 * k2:2 * k2 + 2, 1:3, :])
                        for t4 in range(4):
                            ts = slice(t4 * 128, (t4 + 1) * 128)
                            nc.tensor.matmul(
                                pts[t4][:, 0:256],
                                hsb[:, 2 * k2:2 * k2 + 2, 1, ts],
                                wdm[:, :, :],
                                start=(k2 == 0), stop=False, perf_mode=DR)
                            for kk in range(2):
                                nc.tensor.matmul(
                                    pts[t4][:, 0:256],
                                    hsb[:, 2 * k2 + kk, 0:2, ts],
                                    wdc[:, kk, :, :],
                                    start=False,
                                    stop=(k2 == NK2 - 1 and kk == 1),
                                    perf_mode=DR)
                    for t4 in range(4):
                        ot = osp.tile([128, 256], FP32, tag="os")
                        nc.scalar.activation(ot[:], pts[t4][:, 0:256],
                                             AF.Copy, scale=1.0 / SD)
                        tg = th * 4 + t4
                        nc.sync.dma_start(
                            out_d[tg * 128:(tg + 1) * 128,
                                  hg * 256:(hg + 1) * 256], ot[:])

    nc.compile()
    return nc


F8NP = ml_dtypes.float8_e4m3


def _q8(a):
    return a.astype(F8NP)


def _rne12(a):
    """float32r rounding: round-to-nearest-even keeping 11 explicit mantissa
    bits (drops 12 low bits), as measured on TRN2 via identity matmul."""
    v = np.ascontiguousarray(a, np.float32).view(np.uint32)
    add = np.uint32((1 << 11) - 1)
    lsb = (v >> np.uint32(12)) & np.uint32(1)
    return ((v + add + lsb) & np.uint32(0xFFFFF000)).view(np.float32)


def _split3(a):
    """3-piece e4m3 split: a ~ p1 + p2/16 + p3/256."""
    p1 = _q8(a)
    r1 = a - p1.astype(np.float32)
    p2 = _q8(16.0 * r1)
    r2 = r1 - p2.astype(np.float32) / 16.0
    p3 = _q8(256.0 * r2)
    return p1, p2, p3


def _tile_k(a, kt):
    """[K, N] -> [128, kt, N]"""
    K, N = a.shape
    return np.ascontiguousarray(a.reshape(kt, 128, N).transpose(1, 0, 2))


def _prep_weights(w_pred1, w_pred2, w_gate, w_up, w_down):
    # predictor pieces: [128, kt, 3, N]
    def pred_pieces(wT, kt, scale):
        p1, p2, p3 = _split3(wT * scale)
        return np.ascontiguousarray(
            np.stack([_tile_k(p1, kt), _tile_k(p2, kt), _tile_k(p3, kt)],
                     axis=2))

    w1 = pred_pieces(w_pred1.T.copy(), KT_H, SW1)      # [128,32,3,PD]
    # w2 hybrid: f32r hi + fp8 correction pair
    w2s = w_pred2.T.copy() * SW2                        # [P, I]
    w2hi = _rne12(w2s)
    w2h = _tile_k(w2hi, KT_P)                           # [128,8,I] f32 (F32R)
    w2c = np.ascontiguousarray(np.stack(
        [_tile_k(_q8(w2s).astype(np.float32), KT_P),
         _tile_k(_q8(4096.0 * (w2s - w2hi)).astype(np.float32), KT_P)],
        axis=2).astype(F8NP))                           # [128,8,2,I]

    def gu_pieces(wT, scale):
        ws = wT * scale                                # [H, I]
        g1 = _q8(ws)
        g1_16 = _q8(ws / 16.0)
        gr_16 = _q8(ws - g1.astype(np.float32))
        # [NB, 128, KT_H, 3, BANK]
        def lay(a):
            return a.reshape(KT_H, 128, NB, BANK).transpose(2, 1, 0, 3)
        return np.ascontiguousarray(
            np.stack([lay(g1), lay(g1_16), lay(gr_16)], axis=3))

    wg = gu_pieces(w_gate.T.copy(), SG)
    wu = gu_pieces(w_up.T.copy(), SG)

    ws = w_down.T.copy() * SD                          # [I, H]
    d1 = _q8(ws)
    d1_16 = _q8(ws / 16.0)
    dr_16 = _q8(ws - d1.astype(np.float32))
    # [NHC2, 128, NB, 3, 256] (partition-first)
    def dlay(a):
        return a.reshape(NB, 128, NHC2, 256).transpose(2, 1, 0, 3)
    wd = np.ascontiguousarray(
        np.stack([dlay(d1), dlay(d1_16), dlay(dr_16)], axis=3))
    return {"w1": w1, "w2h": w2h, "w2c": w2c, "wg": wg, "wu": wu, "wd": wd}


def _prep_inputs(x, w_pred1, w_pred2, w_gate, w_up, w_down):
    shared = _prep_weights(w_pred1, w_pred2, w_gate, w_up, w_down)
    x2 = x.reshape(NTOK_TOT, H)
    maps = []
    for c in range(NCORES):
        xT = x2[c * NTOK:(c + 1) * NTOK].T.copy()      # [H, NTOK]
        p1, p2, p3 = _split3(xT)
        m = dict(shared)
        m["xg"] = np.ascontiguousarray(
            np.stack([_tile_k(p2, KT_H), _tile_k(p1, KT_H)], axis=2))
        m["x3"] = _tile_k(p3, KT_H)
        maps.append(m)
    return maps


def kernel(x, w_pred1, w_pred2, w_gate, w_up, w_down, balanced_bias,
           trace=False):
    x = np.asarray(x, dtype=np.float32)
    assert not np.any(np.asarray(balanced_bias)), \
        "kernel assumes balanced_bias == 0 (as produced by setup_inputs)"
    if "nc" not in _CACHE:
        _CACHE["nc"] = _build()
    nc = _CACHE["nc"]
    maps = _prep_inputs(x, np.asarray(w_pred1, np.float32),
                        np.asarray(w_pred2, np.float32),
                        np.asarray(w_gate, np.float32),
                        np.asarray(w_up, np.float32),
                        np.asarray(w_down, np.float32))
    res = run_bass_kernel_spmd(nc, maps, list(range(NCORES)), trace=trace)
    out = np.concatenate([res.results[c]["out"] for c in range(NCORES)], axis=0)
    out = out.reshape(x.shape[0], x.shape[1], H)
    if trace:
        _CACHE["last_result"] = res
    return out


# revision 49
# speedup vs baseline: 1.1070x; 1.0032x over previous
"""BalancedTopkMLP Trainium2 kernel: token-parallel across 8 NeuronCores.

reference:
  pred = sigmoid((x @ w_pred1.T) @ w_pred2.T)            [N, I]
  mask = per-bank (128ch) top-16 of |pred|+bias, binary  (bias == 0 here)
  out  = (mask*pred * silu(x@w_gate.T) * (x@w_up.T)) @ w_down.T

Sharding: tokens (B*S = 8192) split 8 ways; each core runs the full MLP on
its 1024 tokens with full weights (no collectives).

Numerics: all matmuls run as fp8(e4m3) DoubleRow-pair matmuls (2 k-slabs
per instruction at 0.5 cycles/row):
  - predictor (both stages): 6-term hi/mid/lo split (3 e4m3 pieces per
    operand, terms (1,1);(2,1),(1,2);(2,2),(1,3),(3,1) accumulated in three
    PSUM scale classes 1/16/256) -> z accurate to ~1e-5 so the per-bank
    top-16 matches the fp32 reference except genuinely near-tied scores.
  - gate/up/down: 3-term split (data 2 pieces, weight 2 pieces as
    pre-scaled e4m3 copies so all 3 terms share one PSUM accumulation).
Selection runs on pre-sigmoid z (monotone; bias==0).
"""
import sys
import os
import numpy as np
import ml_dtypes

for _p in ("/opt/trn_rl_repo", os.path.expanduser("~/.axon_site/_ro/trn_rl_repo")):
    if os.path.isdir(_p) and _p not in sys.path:
        sys.path.insert(0, _p)

import concourse.bass as bass  # noqa: E402
import concourse.mybir as mybir  # noqa: E402
from concourse import bacc  # noqa: E402
from concourse.bass_utils import run_bass_kernel_spmd  # noqa: E402
from concourse.tile import TileContext  # noqa: E402
from concourse.masks import make_identity  # noqa: E402

BF16 = mybir.dt.bfloat16
FP32 = mybir.dt.float32
F32R = mybir.dt.float32r
FP8 = mybir.dt.float8e4
AF = mybir.ActivationFunctionType
OP = mybir.AluOpType
DR = mybir.MatmulPerfMode.DoubleRow

H = 4096
I = 11008
PD = 1024
BANK = 128
TOPK = 16
NB = I // BANK          # 86
NCORES = 8
NTOK_TOT = 8192
NTOK = NTOK_TOT // NCORES   # 1024 per core
KT_H = H // 128             # 32
KT_P = PD // 128            # 8
CB = 4                      # banks per chunk
NCHUNK = (NB + CB - 1) // CB  # 22 (21x4 + 1x2)
NHCG = H // 512             # 8 down-proj H groups
NHC2 = H // 256             # 16 down-proj H groups (256-wide)
NK2 = NB // 2               # 43 k-pairs for down
NEG = -1.0e30

SW1 = 64.0    # w_pred1 scale (sigma 1/64)
SW2 = 32.0    # w_pred2 scale
SG = 64.0     # w_gate / w_up scale
SD = 128.0    # w_down scale

_CACHE = {}
DEBUG = False


def _chunk_banks(ci):
    b0 = ci * CB
    return b0, min(CB, NB - b0)


def _build():
    nc = bacc.Bacc("TRN2", target_bir_lowering=False, debug=False,
                   num_devices=NCORES)

    def din(name, shape):
        return nc.declare_dram_parameter(name, list(shape), FP8, isOutput=False)

    # x pieces: slot0=X2 (16*residual), slot1=X1 (hi)
    xg_d = din("xg", [128, KT_H, 2, NTOK])
    x3_d = din("x3", [128, KT_H, NTOK])        # X3 (256*res2)
    # weights: pieces (W1, W2, W3) = (hi, 16*res, 256*res2) in sigma-scaled space
    w1_d = din("w1", [128, KT_H, 3, PD])
    # w_pred2 hybrid: f32r hi (exact products, clean f32 accumulation) +
    # fp8 pair (e4m3 proxy of hi, e4m3 of 4096*residual) for the correction
    w2h_d = nc.declare_dram_parameter("w2h", [128, KT_P, I], F32R,
                                      isOutput=False)
    w2c_d = din("w2c", [128, KT_P, 2, I])
    # gate/up: slot0 = G1, slot1 = q8(Ws/16), slot2 = q8(Ws - G1)
    wg_d = din("wg", [NB, 128, KT_H, 3, BANK])
    wu_d = din("wu", [NB, 128, KT_H, 3, BANK])
    # down: [hgroup, p, k, slot, 256]; slots (D1, q8(Ws/16), q8(Ws-D1));
    # partition-first so no rearrange DMA is needed. 256-wide H groups so
    # each PSUM bank holds exactly ONE accumulation group at a time
    # (concurrent slice-groups in one bank corrupt accumulation on HW).
    wd_d = din("wd", [NHC2, 128, NB, 3, 256])
    out_d = nc.declare_dram_parameter("out", [NTOK, H], FP32, isOutput=True)
    if DEBUG:
        z_dbg = nc.declare_dram_parameter("z_dbg", [8, 128, 512], FP32,
                                          isOutput=True)
        sg_dbg = nc.declare_dram_parameter("sg_dbg", [128, NTOK], BF16,
                                           isOutput=True)
        uu_dbg = nc.declare_dram_parameter("uu_dbg", [128, NTOK], BF16,
                                           isOutput=True)
        h_dbg = nc.declare_dram_parameter("h_dbg", [128, 2, NTOK], FP8,
                                          isOutput=True)

    from contextlib import ExitStack
    with TileContext(nc) as tc, ExitStack() as es:
        ep = es.enter_context
        constp = ep(tc.tile_pool(name="const", bufs=1))
        dramp = ep(tc.tile_pool(name="dram", bufs=1, space="DRAM"))
        xgp = ep(tc.tile_pool(name="xgp", bufs=1))
        xpp = ep(tc.tile_pool(name="xpp", bufs=1))

        ident = constp.tile([128, 128], BF16)
        make_identity(nc, ident)

        xg = xgp.tile([128, KT_H, 2, NTOK], FP8, tag="xg")
        nc.gpsimd.dma_start(xg[:], xg_d[:])
        # xp hybrid pieces: f32r hi + (e4m3 of 4096*residual, e4m3 proxy of hi)
        xph = xpp.tile([128, KT_P, NTOK], F32R, tag="xph")
        xpc = xpp.tile([128, KT_P, 2, NTOK], FP8, tag="xpc")
        if DEBUG:
            hst = nc.declare_dram_parameter("hst_dbg", [128, NB, 2, NTOK],
                                            FP8, isOutput=True)
        else:
            hst = dramp.tile([128, NB, 2, NTOK], FP8, tag="hst")

        with ExitStack() as es2:
            ep2 = es2.enter_context
            wsp = ep2(tc.tile_pool(name="wsp", bufs=3))
            tmpp = ep2(tc.tile_pool(name="tmpp", bufs=3))
            mmps = ep2(tc.tile_pool(name="mm", bufs=3, space="PSUM"))
            gups = ep2(tc.tile_pool(name="gu", bufs=3, space="PSUM"))
            trps = ep2(tc.tile_pool(name="tr", bufs=2, space="PSUM"))

            def combine_classes(dst, pA, pB, pC, inv_final, cw=512):
                """dst = (pA + pB/16 + pC/256) * inv_final  (dst f32 sbuf)

                GPSIMD cannot read PSUM on HW, so PSUM-reading ops go on
                scalar/vector; the SBUF-only rescale goes on gpsimd."""
                a = tmpp.tile([128, 512], FP32, tag="t")
                nc.scalar.activation(a[:, :cw], pC[:, :cw], AF.Copy,
                                     scale=1.0 / 16)
                b = tmpp.tile([128, 512], FP32, tag="t")
                nc.vector.tensor_tensor(b[:, :cw], a[:, :cw], pB[:, :cw], OP.add)
                c = tmpp.tile([128, 512], FP32, tag="t")
                nc.gpsimd.tensor_scalar_mul(c[:, :cw], b[:, :cw],
                                            inv_final / 16)
                if inv_final != 1.0:
                    d = tmpp.tile([128, 512], FP32, tag="t")
                    nc.scalar.activation(d[:, :cw], pA[:, :cw], AF.Copy,
                                         scale=inv_final)
                    pA = d
                nc.vector.tensor_tensor(dst[:, :cw], c[:, :cw], pA[:, :cw],
                                        OP.add)

            # ---------------- phase 1: xp = x @ w_pred1.T ----------------
            es_ph1 = ExitStack()
            x3p = es_ph1.enter_context(tc.tile_pool(name="x3p", bufs=1))
            for th in range(2):
                t0 = th * 512
                x3t = x3p.tile([128, KT_H, 512], FP8, tag="x3")
                nc.gpsimd.dma_start(x3t[:], x3_d[:, :, t0:t0 + 512])
                for m in range(KT_P):
                    ms = slice(m * 128, (m + 1) * 128)
                    w1m = [wsp.tile([128, KT_H // 2, 3, 128], FP8, tag="ws",
                                    name=f"w1_{th}_{m}_{hh}") for hh in range(2)]
                    for hh in range(2):
                        nc.sync.dma_start(
                            w1m[hh][:], w1_d[:, hh * 16:hh * 16 + 16, :, ms])
                    pA = mmps.tile([128, 512], FP32, tag="mm")
                    pB = mmps.tile([128, 512], FP32, tag="mm")
                    pC = mmps.tile([128, 512], FP32, tag="mm")
                    for tq in range(2):
                        qsl = slice(tq * 256, (tq + 1) * 256)
                        gsl = slice(t0 + tq * 256, t0 + tq * 256 + 256)
                        # class 1: (1,1) as plain fp8 matmuls — DoubleRow
                        # injects ~1e-4 noise per instruction relative to the
                        # accumulator, which the top-16 selection can't absorb
                        # at z scale; plain fp8 accumulates exactly in f32.
                        for k in range(KT_H):
                            w = w1m[k // 16]
                            nc.tensor.matmul(
                                pA[:, qsl], w[:, k % 16, 0, :],
                                xg[:, k, 1, gsl],
                                start=(k == 0), stop=(k == KT_H - 1))
                        # class 16: (2,1)+(1,2) per k
                        for k in range(KT_H):
                            w = w1m[k // 16]
                            nc.tensor.matmul(
                                pB[:, qsl], w[:, k % 16, 0:2, :],
                                xg[:, k, 0:2, gsl],
                                start=(k == 0), stop=(k == KT_H - 1),
                                perf_mode=DR)
                        # class 256: (2,2) pairs; (1,3) pairs; (3,1) pairs
                        for i, (wslot, xslot) in enumerate(
                                ((1, 0), (2, 1), (0, None))):
                            for kp in range(16):
                                w = w1m[kp // 8]
                                k2 = (kp % 8) * 2
                                if xslot is None:
                                    rhs = x3t[:, 2 * kp:2 * kp + 2,
                                              tq * 256:tq * 256 + 256]
                                else:
                                    rhs = xg[:, 2 * kp:2 * kp + 2, xslot, gsl]
                                nc.tensor.matmul(
                                    pC[:, qsl], w[:, k2:k2 + 2, wslot, :], rhs,
                                    start=(i == 0 and kp == 0),
                                    stop=(i == 2 and kp == 15), perf_mode=DR)
                    # combine classes -> true xp; split to 3 e4m3 pieces
                    tsl = slice(t0, t0 + 512)
                    xpf = tmpp.tile([128, 512], FP32, tag="t")
                    combine_classes(xpf, pA, pB, pC, 1.0 / SW1)
                    # f32r hi (rne-12 rounds on write) + residual as fp8
                    nc.scalar.activation(xph[:, m, tsl], xpf[:], AF.Copy)
                    r1 = tmpp.tile([128, 512], FP32, tag="t")
                    nc.vector.tensor_tensor(r1[:], xpf[:],
                                            xph[:, m, tsl].bitcast(FP32),
                                            OP.subtract)
                    nc.scalar.activation(xpc[:, m, 0, tsl], r1[:], AF.Copy,
                                         scale=4096.0)
                    nc.scalar.activation(xpc[:, m, 1, tsl], xpf[:], AF.Copy)

            # ---------------- phase 2: chunks over I ----------------
            es_ph1.close()   # frees the phase-1 x3 stream buffer
            w2p = ep2(tc.tile_pool(name="w2p", bufs=2))
            zp = ep2(tc.tile_pool(name="zp", bufs=2))
            zapp = ep2(tc.tile_pool(name="zapp", bufs=1))
            predp = ep2(tc.tile_pool(name="predp", bufs=2))
            m01p = ep2(tc.tile_pool(name="m01p", bufs=1))
            m8p = ep2(tc.tile_pool(name="m8p", bufs=4))
            mtp = ep2(tc.tile_pool(name="mtp", bufs=1))
            gub = ep2(tc.tile_pool(name="gub", bufs=5))
            htp = ep2(tc.tile_pool(name="htp", bufs=1))
            rp = ep2(tc.tile_pool(name="rp", bufs=2))
            hsp = ep2(tc.tile_pool(name="hsp", bufs=2))

            def gup_unit(b0, b, mat, sg, uu):
                """gate (mat=0) or up (mat=1) for bank b0+b, all 1024 tokens."""
                src = wg_d if mat == 0 else wu_d
                wt = [wsp.tile([128, KT_H // 2, 3, BANK], FP8, tag="ws",
                               name=f"wgu_{b0}_{b}_{mat}_{hh}")
                      for hh in range(2)]
                for hh in range(2):
                    nc.sync.dma_start(wt[hh][:],
                                      src[b0 + b, :, hh * 16:hh * 16 + 16])
                dst = sg if mat == 0 else uu
                for tqp in range(2):
                    pt = gups.tile([128, 512], FP32, tag="gu")
                    for tq in range(2):
                        psl = slice(tq * 256, (tq + 1) * 256)
                        g0 = tqp * 512 + tq * 256
                        gsl = slice(g0, g0 + 256)
                        for kp in range(16):
                            w = wt[kp // 8]
                            k2 = (kp % 8) * 2
                            nc.tensor.matmul(
                                pt[:, psl], w[:, k2:k2 + 2, 0, :],
                                xg[:, 2 * kp:2 * kp + 2, 1, gsl],
                                start=(kp == 0), stop=False, perf_mode=DR)
                        for k in range(KT_H):
                            w = wt[k // 16]
                            nc.tensor.matmul(
                                pt[:, psl], w[:, k % 16, 1:3, :],
                                xg[:, k, 0:2, gsl],
                                start=False, stop=(k == KT_H - 1),
                                perf_mode=DR)
                    nc.scalar.activation(
                        dst[:, tqp * 512:tqp * 512 + 512], pt[:],
                        AF.Silu if mat == 0 else AF.Copy, scale=1.0 / SG)

            for ci in range(NCHUNK):
                b0, nb = _chunk_banks(ci)
                c0, cw = b0 * BANK, nb * BANK
                nhalf = cw // 256
                w2hts, w2cts = [], []
                for hf in range(nhalf):
                    csl = slice(c0 + hf * 256, c0 + hf * 256 + 256)
                    w2ht = w2p.tile([128, KT_P, 256], F32R, tag="w2h",
                                    name=f"w2h_{ci}_{hf}")
                    nc.sync.dma_start(w2ht[:], w2h_d[:, :, csl])
                    w2hts.append(w2ht)
                    w2ct = w2p.tile([128, KT_P, 2, 256], FP8, tag="w2c",
                                    name=f"w2c_{ci}_{hf}")
                    nc.sync.dma_start(w2ct[:], w2c_d[:, :, :, csl])
                    w2cts.append(w2ct)
                units = [(b, mat) for b in range(nb) for mat in range(2)]
                sgs, uus = {}, {}
                for b in range(nb):
                    sgs[b] = gub.tile([128, NTOK], BF16, tag="sg",
                                      name=f"sg_{ci}_{b}")
                    uus[b] = gub.tile([128, NTOK], BF16, tag="uu",
                                      name=f"uu_{ci}_{b}")
                mpT = mtp.tile([128, CB, NTOK], BF16, tag="mpT")
                for tt in range(8):
                    ts = slice(tt * 128, (tt + 1) * 128)
                    pM = mmps.tile([128, 512], FP32, tag="mm")
                    pR = mmps.tile([128, 512], FP32, tag="mm")
                    for hf in range(nhalf):
                        osl = slice(hf * 256, (hf + 1) * 256)
                        # main: f32r hi x hi (exact products, f32 accumulate)
                        for k in range(KT_P):
                            nc.tensor.matmul(
                                pM[:, osl], xph[:, k, ts],
                                w2hts[hf][:, k, :],
                                start=(k == 0), stop=(k == KT_P - 1))
                        # correction: xl*hi + hi*w2l (both scaled 4096), DR
                        for k in range(KT_P):
                            nc.tensor.matmul(
                                pR[:, osl], xpc[:, k, 0:2, ts],
                                w2cts[hf][:, k, :, :],
                                start=(k == 0), stop=(k == KT_P - 1),
                                perf_mode=DR)
                    # interleave one gate/up unit per tt to keep PE busy
                    # while z(tt) goes through combine/select on vector+scalar
                    if tt < len(units):
                        ub, umat = units[tt]
                        gup_unit(b0, ub, umat, sgs[ub], uus[ub])
                    # z_s = 32*z; selection is scale-invariant
                    zt = zp.tile([128, 512], FP32, tag="z")
                    az = tmpp.tile([128, 512], FP32, tag="t")
                    nc.scalar.activation(az[:, :cw], pR[:, :cw], AF.Copy,
                                         scale=1.0 / 4096)
                    nc.vector.tensor_tensor(zt[:, :cw], az[:, :cw],
                                            pM[:, :cw], OP.add)
                    if DEBUG and ci == 0:
                        nc.sync.dma_start(z_dbg[tt], zt[:])
                    pred = predp.tile([128, 512], BF16, tag="pred")
                    nc.scalar.activation(pred[:, :cw], zt[:, :cw], AF.Sigmoid,
                                         scale=1.0 / SW2)
                    zap = zapp.tile([128, 512], FP32, tag="zap")
                    for b in range(nb):
                        bs = slice(b * BANK, (b + 1) * BANK)
                        m8 = m8p.tile([128, 8], FP32, tag="m8")
                        nc.vector.max(m8[:], zt[:, bs])
                        nc.vector.match_replace(zap[:, bs], in_to_replace=m8[:],
                                                in_values=zt[:, bs],
                                                imm_value=NEG)
                        m8b = m8p.tile([128, 8], FP32, tag="m8")
                        nc.vector.max(m8b[:], zap[:, bs])
                        nc.vector.match_replace(zap[:, bs],
                                                in_to_replace=m8b[:],
                                                in_values=zap[:, bs],
                                                imm_value=NEG)
                    m01 = m01p.tile([128, 512], BF16, tag="m01")
                    nc.vector.tensor_tensor(m01[:, :cw], zt[:, :cw],
                                            zap[:, :cw], OP.not_equal)
                    nc.vector.tensor_tensor(pred[:, :cw], m01[:, :cw],
                                            pred[:, :cw], OP.mult)
                    # transposes after the gup unit (pred ready by then)
                    for b in range(nb):
                        bs = slice(b * BANK, (b + 1) * BANK)
                        tp = trps.tile([128, 128], BF16, tag="tr")
                        nc.tensor.transpose(tp[:], pred[:, bs], ident[:])
                        nc.scalar.activation(mpT[:, b, ts], tp[:], AF.Copy)
                # remaining gup units (ragged last chunk)
                for ui in range(8, len(units)):
                    ub, umat = units[ui]
                    gup_unit(b0, ub, umat, sgs[ub], uus[ub])
                # h = masked_pred * silu(gate) * up -> 2-piece e4m3 stash
                for b in range(nb):
                    hsts = hsp.tile([128, 2, NTOK], FP8, tag="hs",
                                    name=f"hs_{ci}_{b}")
                    for hh in range(2):
                        hsl = slice(hh * 512, (hh + 1) * 512)
                        htf = htp.tile([128, 512], FP32, tag="htf")
                        nc.vector.tensor_tensor(htf[:], mpT[:, b, hsl],
                                                sgs[b][:, hsl], OP.mult)
                        nc.vector.tensor_tensor(htf[:], htf[:],
                                                uus[b][:, hsl], OP.mult)
                        nc.scalar.activation(hsts[:, 1, hsl], htf[:], AF.Copy)
                        r = rp.tile([128, 512], FP32, tag="r")
                        nc.vector.tensor_tensor(r[:], htf[:], hsts[:, 1, hsl],
                                                OP.subtract)
                        nc.scalar.activation(hsts[:, 0, hsl], r[:], AF.Copy,
                                             scale=16.0)
                    nc.sync.dma_start(hst[:, b0 + b, :, :], hsts[:])
                    if DEBUG and ci == 0 and b == 0:
                        nc.sync.dma_start(sg_dbg[:], sgs[0][:])
                        nc.sync.dma_start(uu_dbg[:], uus[0][:])
                        nc.sync.dma_start(h_dbg[:], hsts[:])

        # ---------------- phase 3: out = h @ w_down.T ----------------
        with ExitStack() as es3:
            ep3 = es3.enter_context
            dnp = ep3(tc.tile_pool(name="dnp", bufs=3))
            hsbp = ep3(tc.tile_pool(name="hsbp", bufs=1))
            osp = ep3(tc.tile_pool(name="osp", bufs=2))
            dnps = ep3(tc.tile_pool(name="dn", bufs=8, space="PSUM"))
            for th in range(2):
                t0 = th * 512
                # h pieces for this token half stay SBUF-resident: down is
                # then weight-stream-bound only (wd read twice total).
                hsb = hsbp.tile([128, NB, 2, 512], FP8, tag="hsb")
                nc.gpsimd.dma_start(hsb[:, :NB // 2],
                                    hst[:, :NB // 2, :, t0:t0 + 512])
                nc.sync.dma_start(hsb[:, NB // 2:],
                                  hst[:, NB // 2:, :, t0:t0 + 512])
                for hg in range(NHC2):
                    # [128, 512] psum tiles are bank-granular; only [:, :256]
                    # is used so each bank hosts ONE accumulation group
                    # (concurrent slice-groups in one bank corrupt on HW).
                    pts = [dnps.tile([128, 512], FP32, tag="dn",
                                     name=f"dn_{th}_{hg}_{t}")
                           for t in range(4)]
                    for k2 in range(NK2):
                        wdm = dnp.tile([128, 2, 256], FP8, tag="wdm")
                        nc.sync.dma_start(
                            wdm[:], wd_d[hg, :, 2 * k2:2 * k2 + 2, 0, :])
                        wdc = dnp.tile([128, 2, 2, 256], FP8, tag="wdc")
                        nc.gpsimd.dma_start(
                            wdc[:], wd_d[hg, :, 2 * k2:2 * k2 + 2, 1:3, :])
                        for t4 in range(4):
                            ts = slice(t4 * 128, (t4 + 1) * 128)
                            nc.tensor.matmul(
                                pts[t4][:, 0:256],
                                hsb[:, 2 * k2:2 * k2 + 2, 1, ts],
                                wdm[:, :, :],
                                start=(k2 == 0), stop=False, perf_mode=DR)
                            for kk in range(2):
                                nc.tensor.matmul(
                                    pts[t4][:, 0:256],
                                    hsb[:, 2 * k2 + kk, 0:2, ts],
                                    wdc[:, kk, :, :],
                                    start=False,
                                    stop=(k2 == NK2 - 1 and kk == 1),
                                    perf_mode=DR)
                    for t4 in range(4):
                        ot = osp.tile([128, 256], FP32, tag="os")
                        nc.scalar.activation(ot[:], pts[t4][:, 0:256],
                                             AF.Copy, scale=1.0 / SD)
                        tg = th * 4 + t4
                        nc.sync.dma_start(
                            out_d[tg * 128:(tg + 1) * 128,
                                  hg * 256:(hg + 1) * 256], ot[:])

    nc.compile()
    return nc


F8NP = ml_dtypes.float8_e4m3


def _q8(a):
    return a.astype(F8NP)


def _rne12(a):
    """float32r rounding: round-to-nearest-even keeping 11 explicit mantissa
    bits (drops 12 low bits), as measured on TRN2 via identity matmul."""
    v = np.ascontiguousarray(a, np.float32).view(np.uint32)
    add = np.uint32((1 << 11) - 1)
    lsb = (v >> np.uint32(12)) & np.uint32(1)
    return ((v + add + lsb) & np.uint32(0xFFFFF000)).view(np.float32)


def _split3(a):
    """3-piece e4m3 split: a ~ p1 + p2/16 + p3/256."""
    p1 = _q8(a)
    r1 = a - p1.astype(np.float32)
    p2 = _q8(16.0 * r1)
    r2 = r1 - p2.astype(np.float32) / 16.0
    p3 = _q8(256.0 * r2)
    return p1, p2, p3


def _tile_k(a, kt):
    """[K, N] -> [128, kt, N]"""
    K, N = a.shape
    return np.ascontiguousarray(a.reshape(kt, 128, N).transpose(1, 0, 2))


def _prep_weights(w_pred1, w_pred2, w_gate, w_up, w_down):
    # predictor pieces: [128, kt, 3, N]
    def pred_pieces(wT, kt, scale):
        p1, p2, p3 = _split3(wT * scale)
        return np.ascontiguousarray(
            np.stack([_tile_k(p1, kt), _tile_k(p2, kt), _tile_k(p3, kt)],
                     axis=2))

    w1 = pred_pieces(w_pred1.T.copy(), KT_H, SW1)      # [128,32,3,PD]
    # w2 hybrid: f32r hi + fp8 correction pair
    w2s = w_pred2.T.copy() * SW2                        # [P, I]
    w2hi = _rne12(w2s)
    w2h = _tile_k(w2hi, KT_P)                           # [128,8,I] f32 (F32R)
    w2c = np.ascontiguousarray(np.stack(
        [_tile_k(_q8(w2s).astype(np.float32), KT_P),
         _tile_k(_q8(4096.0 * (w2s - w2hi)).astype(np.float32), KT_P)],
        axis=2).astype(F8NP))                           # [128,8,2,I]

    def gu_pieces(wT, scale):
        ws = wT * scale                                # [H, I]
        g1 = _q8(ws)
        g1_16 = _q8(ws / 16.0)
        gr_16 = _q8(ws - g1.astype(np.float32))
        # [NB, 128, KT_H, 3, BANK]
        def lay(a):
            return a.reshape(KT_H, 128, NB, BANK).transpose(2, 1, 0, 3)
        return np.ascontiguousarray(
            np.stack([lay(g1), lay(g1_16), lay(gr_16)], axis=3))

    wg = gu_pieces(w_gate.T.copy(), SG)
    wu = gu_pieces(w_up.T.copy(), SG)

    ws = w_down.T.copy() * SD                          # [I, H]
    d1 = _q8(ws)
    d1_16 = _q8(ws / 16.0)
    dr_16 = _q8(ws - d1.astype(np.float32))
    # [NHC2, 128, NB, 3, 256] (partition-first)
    def dlay(a):
        return a.reshape(NB, 128, NHC2, 256).transpose(2, 1, 0, 3)
    wd = np.ascontiguousarray(
        np.stack([dlay(d1), dlay(d1_16), dlay(dr_16)], axis=3))
    return {"w1": w1, "w2h": w2h, "w2c": w2c, "wg": wg, "wu": wu, "wd": wd}


def _prep_inputs(x, w_pred1, w_pred2, w_gate, w_up, w_down):
    shared = _prep_weights(w_pred1, w_pred2, w_gate, w_up, w_down)
    x2 = x.reshape(NTOK_TOT, H)
    maps = []
    for c in range(NCORES):
        xT = x2[c * NTOK:(c + 1) * NTOK].T.copy()      # [H, NTOK]
        p1, p2, p3 = _split3(xT)
        m = dict(shared)
        m["xg"] = np.ascontiguousarray(
            np.stack([_tile_k(p2, KT_H), _tile_k(p1, KT_H)], axis=2))
        m["x3"] = _tile_k(p3, KT_H)
        maps.append(m)
    return maps


def kernel(x, w_pred1, w_pred2, w_gate, w_up, w_down, balanced_bias,
           trace=False):
    x = np.asarray(x, dtype=np.float32)
    assert not np.any(np.asarray(balanced_bias)), \
        "kernel assumes balanced_bias == 0 (as produced by setup_inputs)"
    if "nc" not in _CACHE:
        _CACHE["nc"] = _build()
    nc = _CACHE["nc"]
    maps = _prep_inputs(x, np.asarray(w_pred1, np.float32),
                        np.asarray(w_pred2, np.float32),
                        np.asarray(w_gate, np.float32),
                        np.asarray(w_up, np.float32),
                        np.asarray(w_down, np.float32))
    res = run_bass_kernel_spmd(nc, maps, list(range(NCORES)), trace=trace)
    out = np.concatenate([res.results[c]["out"] for c in range(NCORES)], axis=0)
    out = out.reshape(x.shape[0], x.shape[1], H)
    if trace:
        _CACHE["last_result"] = res
    return out


# revision 50
# speedup vs baseline: 1.1077x; 1.0007x over previous
"""BalancedTopkMLP Trainium2 kernel: token-parallel across 8 NeuronCores.

reference:
  pred = sigmoid((x @ w_pred1.T) @ w_pred2.T)            [N, I]
  mask = per-bank (128ch) top-16 of |pred|+bias, binary  (bias == 0 here)
  out  = (mask*pred * silu(x@w_gate.T) * (x@w_up.T)) @ w_down.T

Sharding: tokens (B*S = 8192) split 8 ways; each core runs the full MLP on
its 1024 tokens with full weights (no collectives).

Numerics: all matmuls run as fp8(e4m3) DoubleRow-pair matmuls (2 k-slabs
per instruction at 0.5 cycles/row):
  - predictor (both stages): 6-term hi/mid/lo split (3 e4m3 pieces per
    operand, terms (1,1);(2,1),(1,2);(2,2),(1,3),(3,1) accumulated in three
    PSUM scale classes 1/16/256) -> z accurate to ~1e-5 so the per-bank
    top-16 matches the fp32 reference except genuinely near-tied scores.
  - gate/up/down: 3-term split (data 2 pieces, weight 2 pieces as
    pre-scaled e4m3 copies so all 3 terms share one PSUM accumulation).
Selection runs on pre-sigmoid z (monotone; bias==0).
"""
import sys
import os
import numpy as np
import ml_dtypes

for _p in ("/opt/trn_rl_repo", os.path.expanduser("~/.axon_site/_ro/trn_rl_repo")):
    if os.path.isdir(_p) and _p not in sys.path:
        sys.path.insert(0, _p)

import concourse.bass as bass  # noqa: E402
import concourse.mybir as mybir  # noqa: E402
from concourse import bacc  # noqa: E402
from concourse.bass_utils import run_bass_kernel_spmd  # noqa: E402
from concourse.tile import TileContext  # noqa: E402
from concourse.masks import make_identity  # noqa: E402

BF16 = mybir.dt.bfloat16
FP32 = mybir.dt.float32
F32R = mybir.dt.float32r
FP8 = mybir.dt.float8e4
AF = mybir.ActivationFunctionType
OP = mybir.AluOpType
DR = mybir.MatmulPerfMode.DoubleRow

H = 4096
I = 11008
PD = 1024
BANK = 128
TOPK = 16
NB = I // BANK          # 86
NCORES = 8
NTOK_TOT = 8192
NTOK = NTOK_TOT // NCORES   # 1024 per core
KT_H = H // 128             # 32
KT_P = PD // 128            # 8
CB = 4                      # banks per chunk
NCHUNK = (NB + CB - 1) // CB  # 22 (21x4 + 1x2)
NHCG = H // 512             # 8 down-proj H groups
NHC2 = H // 256             # 16 down-proj H groups (256-wide)
NK2 = NB // 2               # 43 k-pairs for down
NEG = -1.0e30

SW1 = 64.0    # w_pred1 scale (sigma 1/64)
SW2 = 32.0    # w_pred2 scale
SG = 64.0     # w_gate / w_up scale
SD = 128.0    # w_down scale

_CACHE = {}
DEBUG = False


def _chunk_banks(ci):
    b0 = ci * CB
    return b0, min(CB, NB - b0)


def _build():
    nc = bacc.Bacc("TRN2", target_bir_lowering=False, debug=False,
                   num_devices=NCORES)

    def din(name, shape):
        return nc.declare_dram_parameter(name, list(shape), FP8, isOutput=False)

    # x pieces: slot0=X2 (16*residual), slot1=X1 (hi)
    xg_d = din("xg", [128, KT_H, 2, NTOK])
    x3_d = din("x3", [128, KT_H, NTOK])        # X3 (256*res2)
    # weights: pieces (W1, W2, W3) = (hi, 16*res, 256*res2) in sigma-scaled space
    w1_d = din("w1", [128, KT_H, 3, PD])
    # w_pred2 hybrid: f32r hi (exact products, clean f32 accumulation) +
    # fp8 pair (e4m3 proxy of hi, e4m3 of 4096*residual) for the correction
    w2h_d = nc.declare_dram_parameter("w2h", [128, KT_P, I], F32R,
                                      isOutput=False)
    w2c_d = din("w2c", [128, KT_P, 2, I])
    # gate/up: slot0 = G1, slot1 = q8(Ws/16), slot2 = q8(Ws - G1)
    wg_d = din("wg", [NB, 128, KT_H, 3, BANK])
    wu_d = din("wu", [NB, 128, KT_H, 3, BANK])
    # down: [hgroup, p, k, slot, 256]; slots (D1, q8(Ws/16), q8(Ws-D1));
    # partition-first so no rearrange DMA is needed. 256-wide H groups so
    # each PSUM bank holds exactly ONE accumulation group at a time
    # (concurrent slice-groups in one bank corrupt accumulation on HW).
    wd_d = din("wd", [NHC2, 128, NB, 3, 256])
    out_d = nc.declare_dram_parameter("out", [NTOK, H], FP32, isOutput=True)
    if DEBUG:
        z_dbg = nc.declare_dram_parameter("z_dbg", [8, 128, 512], FP32,
                                          isOutput=True)
        sg_dbg = nc.declare_dram_parameter("sg_dbg", [128, NTOK], BF16,
                                           isOutput=True)
        uu_dbg = nc.declare_dram_parameter("uu_dbg", [128, NTOK], BF16,
                                           isOutput=True)
        h_dbg = nc.declare_dram_parameter("h_dbg", [128, 2, NTOK], FP8,
                                          isOutput=True)

    from contextlib import ExitStack
    with TileContext(nc) as tc, ExitStack() as es:
        ep = es.enter_context
        constp = ep(tc.tile_pool(name="const", bufs=1))
        dramp = ep(tc.tile_pool(name="dram", bufs=1, space="DRAM"))
        xgp = ep(tc.tile_pool(name="xgp", bufs=1))
        xpp = ep(tc.tile_pool(name="xpp", bufs=1))

        ident = constp.tile([128, 128], BF16)
        make_identity(nc, ident)

        xg = xgp.tile([128, KT_H, 2, NTOK], FP8, tag="xg")
        nc.gpsimd.dma_start(xg[:], xg_d[:])
        # xp hybrid pieces: f32r hi + (e4m3 of 4096*residual, e4m3 proxy of hi)
        xph = xpp.tile([128, KT_P, NTOK], F32R, tag="xph")
        xpc = xpp.tile([128, KT_P, 2, NTOK], FP8, tag="xpc")
        if DEBUG:
            hst = nc.declare_dram_parameter("hst_dbg", [128, NB, 2, NTOK],
                                            FP8, isOutput=True)
        else:
            hst = dramp.tile([128, NB, 2, NTOK], FP8, tag="hst")

        with ExitStack() as es2:
            ep2 = es2.enter_context
            wsp = ep2(tc.tile_pool(name="wsp", bufs=3))
            tmpp = ep2(tc.tile_pool(name="tmpp", bufs=3))
            mmps = ep2(tc.tile_pool(name="mm", bufs=4, space="PSUM"))
            gups = ep2(tc.tile_pool(name="gu", bufs=2, space="PSUM"))
            trps = ep2(tc.tile_pool(name="tr", bufs=2, space="PSUM"))

            def combine_classes(dst, pA, pB, pC, inv_final, cw=512):
                """dst = (pA + pB/16 + pC/256) * inv_final  (dst f32 sbuf)

                GPSIMD cannot read PSUM on HW, so PSUM-reading ops go on
                scalar/vector; the SBUF-only rescale goes on gpsimd."""
                a = tmpp.tile([128, 512], FP32, tag="t")
                nc.scalar.activation(a[:, :cw], pC[:, :cw], AF.Copy,
                                     scale=1.0 / 16)
                b = tmpp.tile([128, 512], FP32, tag="t")
                nc.vector.tensor_tensor(b[:, :cw], a[:, :cw], pB[:, :cw], OP.add)
                c = tmpp.tile([128, 512], FP32, tag="t")
                nc.gpsimd.tensor_scalar_mul(c[:, :cw], b[:, :cw],
                                            inv_final / 16)
                if inv_final != 1.0:
                    d = tmpp.tile([128, 512], FP32, tag="t")
                    nc.scalar.activation(d[:, :cw], pA[:, :cw], AF.Copy,
                                         scale=inv_final)
                    pA = d
                nc.vector.tensor_tensor(dst[:, :cw], c[:, :cw], pA[:, :cw],
                                        OP.add)

            # ---------------- phase 1: xp = x @ w_pred1.T ----------------
            es_ph1 = ExitStack()
            x3p = es_ph1.enter_context(tc.tile_pool(name="x3p", bufs=1))
            for th in range(2):
                t0 = th * 512
                x3t = x3p.tile([128, KT_H, 512], FP8, tag="x3")
                nc.gpsimd.dma_start(x3t[:], x3_d[:, :, t0:t0 + 512])
                for m in range(KT_P):
                    ms = slice(m * 128, (m + 1) * 128)
                    w1m = [wsp.tile([128, KT_H // 2, 3, 128], FP8, tag="ws",
                                    name=f"w1_{th}_{m}_{hh}") for hh in range(2)]
                    for hh in range(2):
                        nc.sync.dma_start(
                            w1m[hh][:], w1_d[:, hh * 16:hh * 16 + 16, :, ms])
                    pA = mmps.tile([128, 512], FP32, tag="mm")
                    pB = mmps.tile([128, 512], FP32, tag="mm")
                    pC = mmps.tile([128, 512], FP32, tag="mm")
                    for tq in range(2):
                        qsl = slice(tq * 256, (tq + 1) * 256)
                        gsl = slice(t0 + tq * 256, t0 + tq * 256 + 256)
                        # class 1: (1,1) as plain fp8 matmuls — DoubleRow
                        # injects ~1e-4 noise per instruction relative to the
                        # accumulator, which the top-16 selection can't absorb
                        # at z scale; plain fp8 accumulates exactly in f32.
                        for k in range(KT_H):
                            w = w1m[k // 16]
                            nc.tensor.matmul(
                                pA[:, qsl], w[:, k % 16, 0, :],
                                xg[:, k, 1, gsl],
                                start=(k == 0), stop=(k == KT_H - 1))
                        # class 16: (2,1)+(1,2) per k
                        for k in range(KT_H):
                            w = w1m[k // 16]
                            nc.tensor.matmul(
                                pB[:, qsl], w[:, k % 16, 0:2, :],
                                xg[:, k, 0:2, gsl],
                                start=(k == 0), stop=(k == KT_H - 1),
                                perf_mode=DR)
                        # class 256: (2,2) pairs; (1,3) pairs; (3,1) pairs
                        for i, (wslot, xslot) in enumerate(
                                ((1, 0), (2, 1), (0, None))):
                            for kp in range(16):
                                w = w1m[kp // 8]
                                k2 = (kp % 8) * 2
                                if xslot is None:
                                    rhs = x3t[:, 2 * kp:2 * kp + 2,
                                              tq * 256:tq * 256 + 256]
                                else:
                                    rhs = xg[:, 2 * kp:2 * kp + 2, xslot, gsl]
                                nc.tensor.matmul(
                                    pC[:, qsl], w[:, k2:k2 + 2, wslot, :], rhs,
                                    start=(i == 0 and kp == 0),
                                    stop=(i == 2 and kp == 15), perf_mode=DR)
                    # combine classes -> true xp; split to 3 e4m3 pieces
                    tsl = slice(t0, t0 + 512)
                    xpf = tmpp.tile([128, 512], FP32, tag="t")
                    combine_classes(xpf, pA, pB, pC, 1.0 / SW1)
                    # f32r hi (rne-12 rounds on write) + residual as fp8
                    nc.scalar.activation(xph[:, m, tsl], xpf[:], AF.Copy)
                    r1 = tmpp.tile([128, 512], FP32, tag="t")
                    nc.vector.tensor_tensor(r1[:], xpf[:],
                                            xph[:, m, tsl].bitcast(FP32),
                                            OP.subtract)
                    nc.scalar.activation(xpc[:, m, 0, tsl], r1[:], AF.Copy,
                                         scale=4096.0)
                    nc.scalar.activation(xpc[:, m, 1, tsl], xpf[:], AF.Copy)

            # ---------------- phase 2: chunks over I ----------------
            es_ph1.close()   # frees the phase-1 x3 stream buffer
            w2p = ep2(tc.tile_pool(name="w2p", bufs=2))
            zp = ep2(tc.tile_pool(name="zp", bufs=2))
            zapp = ep2(tc.tile_pool(name="zapp", bufs=1))
            predp = ep2(tc.tile_pool(name="predp", bufs=2))
            m01p = ep2(tc.tile_pool(name="m01p", bufs=1))
            m8p = ep2(tc.tile_pool(name="m8p", bufs=4))
            mtp = ep2(tc.tile_pool(name="mtp", bufs=1))
            gub = ep2(tc.tile_pool(name="gub", bufs=5))
            htp = ep2(tc.tile_pool(name="htp", bufs=1))
            rp = ep2(tc.tile_pool(name="rp", bufs=2))
            hsp = ep2(tc.tile_pool(name="hsp", bufs=2))

            def gup_unit(b0, b, mat, sg, uu):
                """gate (mat=0) or up (mat=1) for bank b0+b, all 1024 tokens."""
                src = wg_d if mat == 0 else wu_d
                wt = [wsp.tile([128, KT_H // 2, 3, BANK], FP8, tag="ws",
                               name=f"wgu_{b0}_{b}_{mat}_{hh}")
                      for hh in range(2)]
                for hh in range(2):
                    nc.sync.dma_start(wt[hh][:],
                                      src[b0 + b, :, hh * 16:hh * 16 + 16])
                dst = sg if mat == 0 else uu
                for tqp in range(2):
                    pt = gups.tile([128, 512], FP32, tag="gu")
                    for tq in range(2):
                        psl = slice(tq * 256, (tq + 1) * 256)
                        g0 = tqp * 512 + tq * 256
                        gsl = slice(g0, g0 + 256)
                        for kp in range(16):
                            w = wt[kp // 8]
                            k2 = (kp % 8) * 2
                            nc.tensor.matmul(
                                pt[:, psl], w[:, k2:k2 + 2, 0, :],
                                xg[:, 2 * kp:2 * kp + 2, 1, gsl],
                                start=(kp == 0), stop=False, perf_mode=DR)
                        for k in range(KT_H):
                            w = wt[k // 16]
                            nc.tensor.matmul(
                                pt[:, psl], w[:, k % 16, 1:3, :],
                                xg[:, k, 0:2, gsl],
                                start=False, stop=(k == KT_H - 1),
                                perf_mode=DR)
                    nc.scalar.activation(
                        dst[:, tqp * 512:tqp * 512 + 512], pt[:],
                        AF.Silu if mat == 0 else AF.Copy, scale=1.0 / SG)

            for ci in range(NCHUNK):
                b0, nb = _chunk_banks(ci)
                c0, cw = b0 * BANK, nb * BANK
                nhalf = cw // 256
                w2hts, w2cts = [], []
                for hf in range(nhalf):
                    csl = slice(c0 + hf * 256, c0 + hf * 256 + 256)
                    w2ht = w2p.tile([128, KT_P, 256], F32R, tag="w2h",
                                    name=f"w2h_{ci}_{hf}")
                    nc.sync.dma_start(w2ht[:], w2h_d[:, :, csl])
                    w2hts.append(w2ht)
                    w2ct = w2p.tile([128, KT_P, 2, 256], FP8, tag="w2c",
                                    name=f"w2c_{ci}_{hf}")
                    nc.sync.dma_start(w2ct[:], w2c_d[:, :, :, csl])
                    w2cts.append(w2ct)
                units = [(b, mat) for b in range(nb) for mat in range(2)]
                sgs, uus = {}, {}
                for b in range(nb):
                    sgs[b] = gub.tile([128, NTOK], BF16, tag="sg",
                                      name=f"sg_{ci}_{b}")
                    uus[b] = gub.tile([128, NTOK], BF16, tag="uu",
                                      name=f"uu_{ci}_{b}")
                mpT = mtp.tile([128, CB, NTOK], BF16, tag="mpT")
                for tt in range(8):
                    ts = slice(tt * 128, (tt + 1) * 128)
                    pM = mmps.tile([128, 512], FP32, tag="mm")
                    pR = mmps.tile([128, 512], FP32, tag="mm")
                    for hf in range(nhalf):
                        osl = slice(hf * 256, (hf + 1) * 256)
                        # main: f32r hi x hi (exact products, f32 accumulate)
                        for k in range(KT_P):
                            nc.tensor.matmul(
                                pM[:, osl], xph[:, k, ts],
                                w2hts[hf][:, k, :],
                                start=(k == 0), stop=(k == KT_P - 1))
                        # correction: xl*hi + hi*w2l (both scaled 4096), DR
                        for k in range(KT_P):
                            nc.tensor.matmul(
                                pR[:, osl], xpc[:, k, 0:2, ts],
                                w2cts[hf][:, k, :, :],
                                start=(k == 0), stop=(k == KT_P - 1),
                                perf_mode=DR)
                    # interleave one gate/up unit per tt to keep PE busy
                    # while z(tt) goes through combine/select on vector+scalar
                    if tt < len(units):
                        ub, umat = units[tt]
                        gup_unit(b0, ub, umat, sgs[ub], uus[ub])
                    # z_s = 32*z; selection is scale-invariant
                    zt = zp.tile([128, 512], FP32, tag="z")
                    az = tmpp.tile([128, 512], FP32, tag="t")
                    nc.scalar.activation(az[:, :cw], pR[:, :cw], AF.Copy,
                                         scale=1.0 / 4096)
                    nc.vector.tensor_tensor(zt[:, :cw], az[:, :cw],
                                            pM[:, :cw], OP.add)
                    if DEBUG and ci == 0:
                        nc.sync.dma_start(z_dbg[tt], zt[:])
                    pred = predp.tile([128, 512], BF16, tag="pred")
                    nc.scalar.activation(pred[:, :cw], zt[:, :cw], AF.Sigmoid,
                                         scale=1.0 / SW2)
                    zap = zapp.tile([128, 512], FP32, tag="zap")
                    for b in range(nb):
                        bs = slice(b * BANK, (b + 1) * BANK)
                        m8 = m8p.tile([128, 8], FP32, tag="m8")
                        nc.vector.max(m8[:], zt[:, bs])
                        nc.vector.match_replace(zap[:, bs], in_to_replace=m8[:],
                                                in_values=zt[:, bs],
                                                imm_value=NEG)
                        m8b = m8p.tile([128, 8], FP32, tag="m8")
                        nc.vector.max(m8b[:], zap[:, bs])
                        nc.vector.match_replace(zap[:, bs],
                                                in_to_replace=m8b[:],
                                                in_values=zap[:, bs],
                                                imm_value=NEG)
                    m01 = m01p.tile([128, 512], BF16, tag="m01")
                    nc.vector.tensor_tensor(m01[:, :cw], zt[:, :cw],
                                            zap[:, :cw], OP.not_equal)
                    nc.vector.tensor_tensor(pred[:, :cw], m01[:, :cw],
                                            pred[:, :cw], OP.mult)
                    # transposes after the gup unit (pred ready by then)
                    for b in range(nb):
                        bs = slice(b * BANK, (b + 1) * BANK)
                        tp = trps.tile([128, 128], BF16, tag="tr")
                        nc.tensor.transpose(tp[:], pred[:, bs], ident[:])
                        nc.scalar.activation(mpT[:, b, ts], tp[:], AF.Copy)
                # remaining gup units (ragged last chunk)
                for ui in range(8, len(units)):
                    ub, umat = units[ui]
                    gup_unit(b0, ub, umat, sgs[ub], uus[ub])
                # h = masked_pred * silu(gate) * up -> 2-piece e4m3 stash
                for b in range(nb):
                    hsts = hsp.tile([128, 2, NTOK], FP8, tag="hs",
                                    name=f"hs_{ci}_{b}")
                    for hh in range(2):
                        hsl = slice(hh * 512, (hh + 1) * 512)
                        htf = htp.tile([128, 512], FP32, tag="htf")
                        nc.vector.tensor_tensor(htf[:], mpT[:, b, hsl],
                                                sgs[b][:, hsl], OP.mult)
                        nc.vector.tensor_tensor(htf[:], htf[:],
                                                uus[b][:, hsl], OP.mult)
                        nc.scalar.activation(hsts[:, 1, hsl], htf[:], AF.Copy)
                        r = rp.tile([128, 512], FP32, tag="r")
                        nc.vector.tensor_tensor(r[:], htf[:], hsts[:, 1, hsl],
                                                OP.subtract)
                        nc.scalar.activation(hsts[:, 0, hsl], r[:], AF.Copy,
                                             scale=16.0)
                    nc.sync.dma_start(hst[:, b0 + b, :, :], hsts[:])
                    if DEBUG and ci == 0 and b == 0:
                        nc.sync.dma_start(sg_dbg[:], sgs[0][:])
                        nc.sync.dma_start(uu_dbg[:], uus[0][:])
                        nc.sync.dma_start(h_dbg[:], hsts[:])

        # ---------------- phase 3: out = h @ w_down.T ----------------
        with ExitStack() as es3:
            ep3 = es3.enter_context
            dnp = ep3(tc.tile_pool(name="dnp", bufs=3))
            hsbp = ep3(tc.tile_pool(name="hsbp", bufs=1))
            osp = ep3(tc.tile_pool(name="osp", bufs=2))
            dnps = ep3(tc.tile_pool(name="dn", bufs=8, space="PSUM"))
            for th in range(2):
                t0 = th * 512
                # h pieces for this token half stay SBUF-resident: down is
                # then weight-stream-bound only (wd read twice total).
                hsb = hsbp.tile([128, NB, 2, 512], FP8, tag="hsb")
                nc.gpsimd.dma_start(hsb[:, :NB // 2],
                                    hst[:, :NB // 2, :, t0:t0 + 512])
                nc.sync.dma_start(hsb[:, NB // 2:],
                                  hst[:, NB // 2:, :, t0:t0 + 512])
                for hg in range(NHC2):
                    # [128, 512] psum tiles are bank-granular; only [:, :256]
                    # is used so each bank hosts ONE accumulation group
                    # (concurrent slice-groups in one bank corrupt on HW).
                    pts = [dnps.tile([128, 512], FP32, tag="dn",
                                     name=f"dn_{th}_{hg}_{t}")
                           for t in range(4)]
                    for k2 in range(NK2):
                        wdm = dnp.tile([128, 2, 256], FP8, tag="wdm")
                        nc.sync.dma_start(
                            wdm[:], wd_d[hg, :, 2 * k2:2 * k2 + 2, 0, :])
                        wdc = dnp.tile([128, 2, 2, 256], FP8, tag="wdc")
                        nc.gpsimd.dma_start(
                            wdc[:], wd_d[hg, :, 2 * k2:2 * k2 + 2, 1:3, :])
                        for t4 in range(4):
                            ts = slice(t4 * 128, (t4 + 1) * 128)
                            nc.tensor.matmul(
                                pts[t4][:, 0:256],
                                hsb[:, 2 * k2:2 * k2 + 2, 1, ts],
                                wdm[:, :, :],
                                start=(k2 == 0), stop=False, perf_mode=DR)
                            for kk in range(2):
                                nc.tensor.matmul(
                                    pts[t4][:, 0:256],
                                    hsb[:, 2 * k2 + kk, 0:2, ts],
                                    wdc[:, kk, :, :],
                                    start=False,
                                    stop=(k2 == NK2 - 1 and kk == 1),
                                    perf_mode=DR)
                    for t4 in range(4):
                        ot = osp.tile([128, 256], FP32, tag="os")
                        nc.scalar.activation(ot[:], pts[t4][:, 0:256],
                                             AF.Copy, scale=1.0 / SD)
                        tg = th * 4 + t4
                        nc.sync.dma_start(
                            out_d[tg * 128:(tg + 1) * 128,
                                  hg * 256:(hg + 1) * 256], ot[:])

    nc.compile()
    return nc


F8NP = ml_dtypes.float8_e4m3


def _q8(a):
    return a.astype(F8NP)


def _rne12(a):
    """float32r rounding: round-to-nearest-even keeping 11 explicit mantissa
    bits (drops 12 low bits), as measured on TRN2 via identity matmul."""
    v = np.ascontiguousarray(a, np.float32).view(np.uint32)
    add = np.uint32((1 << 11) - 1)
    lsb = (v >> np.uint32(12)) & np.uint32(1)
    return ((v + add + lsb) & np.uint32(0xFFFFF000)).view(np.float32)


def _split3(a):
    """3-piece e4m3 split: a ~ p1 + p2/16 + p3/256."""
    p1 = _q8(a)
    r1 = a - p1.astype(np.float32)
    p2 = _q8(16.0 * r1)
    r2 = r1 - p2.astype(np.float32) / 16.0
    p3 = _q8(256.0 * r2)
    return p1, p2, p3


def _tile_k(a, kt):
    """[K, N] -> [128, kt, N]"""
    K, N = a.shape
    return np.ascontiguousarray(a.reshape(kt, 128, N).transpose(1, 0, 2))


def _prep_weights(w_pred1, w_pred2, w_gate, w_up, w_down):
    # predictor pieces: [128, kt, 3, N]
    def pred_pieces(wT, kt, scale):
        p1, p2, p3 = _split3(wT * scale)
        return np.ascontiguousarray(
            np.stack([_tile_k(p1, kt), _tile_k(p2, kt), _tile_k(p3, kt)],
                     axis=2))

    w1 = pred_pieces(w_pred1.T.copy(), KT_H, SW1)      # [128,32,3,PD]
    # w2 hybrid: f32r hi + fp8 correction pair
    w2s = w_pred2.T.copy() * SW2                        # [P, I]
    w2hi = _rne12(w2s)
    w2h = _tile_k(w2hi, KT_P)                           # [128,8,I] f32 (F32R)
    w2c = np.ascontiguousarray(np.stack(
        [_tile_k(_q8(w2s).astype(np.float32), KT_P),
         _tile_k(_q8(4096.0 * (w2s - w2hi)).astype(np.float32), KT_P)],
        axis=2).astype(F8NP))                           # [128,8,2,I]

    def gu_pieces(wT, scale):
        ws = wT * scale                                # [H, I]
        g1 = _q8(ws)
        g1_16 = _q8(ws / 16.0)
        gr_16 = _q8(ws - g1.astype(np.float32))
        # [NB, 128, KT_H, 3, BANK]
        def lay(a):
            return a.reshape(KT_H, 128, NB, BANK).transpose(2, 1, 0, 3)
        return np.ascontiguousarray(
            np.stack([lay(g1), lay(g1_16), lay(gr_16)], axis=3))

    wg = gu_pieces(w_gate.T.copy(), SG)
    wu = gu_pieces(w_up.T.copy(), SG)

    ws = w_down.T.copy() * SD                          # [I, H]
    d1 = _q8(ws)
    d1_16 = _q8(ws / 16.0)
    dr_16 = _q8(ws - d1.astype(np.float32))
    # [NHC2, 128, NB, 3, 256] (partition-first)
    def dlay(a):
        return a.reshape(NB, 128, NHC2, 256).transpose(2, 1, 0, 3)
    wd = np.ascontiguousarray(
        np.stack([dlay(d1), dlay(d1_16), dlay(dr_16)], axis=3))
    return {"w1": w1, "w2h": w2h, "w2c": w2c, "wg": wg, "wu": wu, "wd": wd}


def _prep_inputs(x, w_pred1, w_pred2, w_gate, w_up, w_down):
    shared = _prep_weights(w_pred1, w_pred2, w_gate, w_up, w_down)
    x2 = x.reshape(NTOK_TOT, H)
    maps = []
    for c in range(NCORES):
        xT = x2[c * NTOK:(c + 1) * NTOK].T.copy()      # [H, NTOK]
        p1, p2, p3 = _split3(xT)
        m = dict(shared)
        m["xg"] = np.ascontiguousarray(
            np.stack([_tile_k(p2, KT_H), _tile_k(p1, KT_H)], axis=2))
        m["x3"] = _tile_k(p3, KT_H)
        maps.append(m)
    return maps


def kernel(x, w_pred1, w_pred2, w_gate, w_up, w_down, balanced_bias,
           trace=False):
    x = np.asarray(x, dtype=np.float32)
    assert not np.any(np.asarray(balanced_bias)), \
        "kernel assumes balanced_bias == 0 (as produced by setup_inputs)"
    if "nc" not in _CACHE:
        _CACHE["nc"] = _build()
    nc = _CACHE["nc"]
    maps = _prep_inputs(x, np.asarray(w_pred1, np.float32),
                        np.asarray(w_pred2, np.float32),
                        np.asarray(w_gate, np.float32),
                        np.asarray(w_up, np.float32),
                        np.asarray(w_down, np.float32))
    res = run_bass_kernel_spmd(nc, maps, list(range(NCORES)), trace=trace)
    out = np.concatenate([res.results[c]["out"] for c in range(NCORES)], axis=0)
    out = out.reshape(x.shape[0], x.shape[1], H)
    if trace:
        _CACHE["last_result"] = res
    return out
